# revision 1
# baseline (speedup 1.0000x reference)
"""Trainium2 Bass kernel for the 2-layer heterogeneous GNN (GATv2 + CGConv).

Sharding: destination nodes (both node types) are split into 8 contiguous
ranges of 2560 (N padded 20000 -> 20480); each core owns the edges that
target its range, for all 4 relations.  Node features are replicated
(SBUF-resident, bf16, node-wrapped layout) so per-edge gathers are local
SBUF->SBUF dma_gather ops; the one inter-layer halo exchange is a single
AllGather of the updated 2560-row slices.

Per relation, per dst tile (128 nodes), edges are packed into fixed blocks
of 128 (padded with a dummy node).  Per-edge transforms run on the PE
(gathered features feature-major as the stationary operand, weights
streaming), and segment softmax / segment sums are one-hot selector
matmuls (selectors built on-device with is_equal against an iota row).
"""

import os
import numpy as np
import ml_dtypes

BF = ml_dtypes.bfloat16

N = 20000
D = 128
H = 4
L = 2
E = 80000
CORES = 8
NPAD = 20480
SHARD = 2560
TILES = 20          # dst tiles of 128 per core
RANKS = NPAD // 128  # 160
PAD_NODE = 20000    # zero-feature padding node (valid gather target)

LAST_EXEC_NS = None

# relation table: (name, kind, src_type, dst_type)
RELS = [
    ("loses", "cg", "my", "opp"),
    ("beats", "gat", "my", "opp"),
    ("rev_beats", "cg", "opp", "my"),
    ("rev_loses", "gat", "opp", "my"),
]


# ----------------------------------------------------------------- host prep

def _wrap_nodes(x):
    """[N,128] f32 -> node-wrapped [128, RANKS*128] bf16 (node n at
    partition n%128, cols (n//128)*128 : +128)."""
    xp = np.zeros((NPAD, D), np.float32)
    xp[:N] = x
    return np.ascontiguousarray(
        xp.reshape(RANKS, 128, D).transpose(1, 0, 2).reshape(128, RANKS * D)
    ).astype(BF)


def _dst_major_slice(x, c):
    """core c's own dst slice, dst-major [128, TILES*128] bf16."""
    xp = np.zeros((NPAD, D), np.float32)
    xp[:N] = x
    sl = xp[c * SHARD:(c + 1) * SHARD]
    return np.ascontiguousarray(
        sl.reshape(TILES, 128, D).transpose(1, 0, 2).reshape(128, TILES * D)
    ).astype(BF)


def _prep_edges(ei):
    """bucket edges by (core, dst tile); returns per-core lists + max tile count."""
    src = np.asarray(ei[0]).astype(np.int64)
    dst = np.asarray(ei[1]).astype(np.int64)
    percore = []
    maxcnt = 1
    for c in range(CORES):
        m = (dst >= c * SHARD) & (dst < (c + 1) * SHARD)
        s, d = src[m], dst[m]
        dl = d - c * SHARD
        tid = dl // 128
        buckets = [np.nonzero(tid == t)[0] for t in range(TILES)]
        for b in buckets:
            maxcnt = max(maxcnt, len(b))
        percore.append((s, d, dl, buckets))
    return percore, maxcnt


def _pack_edges(percore, Bmax):
    """-> per-core (src_ids [EP], dst_ids [EP], dloc [EP]) with per-tile padding."""
    out = []
    for (s, d, dl, buckets) in percore:
        src_a = np.full((TILES, Bmax * 128), PAD_NODE, np.int64)
        dst_a = np.full((TILES, Bmax * 128), PAD_NODE, np.int64)
        loc_a = np.full((TILES, Bmax * 128), -1.0, np.float32)
        for t, b in enumerate(buckets):
            n = len(b)
            src_a[t, :n] = s[b]
            dst_a[t, :n] = d[b]
            loc_a[t, :n] = (dl[b] % 128).astype(np.float32)
        out.append((src_a.reshape(-1), dst_a.reshape(-1), loc_a.reshape(-1)))
    return out


def _idx_dev(a):
    """[EP] int -> [128, EP//16] int16 (16-partition wrap, replicated 8x)."""
    x = a.astype(np.int16).reshape(-1, 16).T
    return np.ascontiguousarray(np.tile(x, (8, 1)))


def _loc_dev(a):
    """[EP] f32 -> [128, EP//128] (edge e at [e%128, e//128])."""
    return np.ascontiguousarray(a.reshape(-1, 128).T.astype(np.float32))


def _rep(v, rows=128):
    """replicate a [K] vector across partitions -> [rows, K]."""
    return np.ascontiguousarray(np.tile(np.asarray(v, np.float32).reshape(1, -1), (rows, 1)))


# ------------------------------------------------------------- program build

def _build_program(Bmax):
    import concourse.bass as bass
    import concourse.bacc as bacc
    import concourse.mybir as mybir
    import concourse.tile as tile

    F32, BF16, I16 = mybir.dt.float32, mybir.dt.bfloat16, mybir.dt.int16
    AF = mybir.ActivationFunctionType
    OP = mybir.AluOpType

    EP = TILES * Bmax * 128
    GTILES = 4                      # dst tiles per gather chunk
    EPQ = GTILES * Bmax * 128       # idxs per gather chunk
    NCH = TILES // GTILES           # gather chunks per relation side

    # dev bisection knobs (default = full kernel)
    k_rels = os.environ.get("K_RELS", "")
    k_layers = int(os.environ.get("K_LAYERS", str(L)))
    k_ag = os.environ.get("K_AG", "1") == "1"
    k_stage = os.environ.get("K_STAGE", "full")   # gather|mm|act|full
    k_epi = os.environ.get("K_EPI", "1") == "1"
    rels_active = [r for r in RELS if (not k_rels or r[0] in k_rels.split(","))]

    nc = bacc.Bacc("TRN2", target_bir_lowering=False, debug=False,
                   num_devices=CORES)

    dr = {}
    dr["xw_my"] = nc.dram_tensor("xw_my", [128, RANKS * D], BF16, kind="ExternalInput")
    dr["xw_opp"] = nc.dram_tensor("xw_opp", [128, RANKS * D], BF16, kind="ExternalInput")
    dr["xres_my"] = nc.dram_tensor("xres_my", [128, TILES * D], BF16, kind="ExternalInput")
    dr["xres_opp"] = nc.dram_tensor("xres_opp", [128, TILES * D], BF16, kind="ExternalInput")
    for rname, kind, _, _ in RELS:
        dr[f"si_{rname}"] = nc.dram_tensor(f"si_{rname}", [128, EP // 16], I16, kind="ExternalInput")
        dr[f"di_{rname}"] = nc.dram_tensor(f"di_{rname}", [128, EP // 16], I16, kind="ExternalInput")
        dr[f"dl_{rname}"] = nc.dram_tensor(f"dl_{rname}", [128, EP // 128], F32, kind="ExternalInput")
        if kind == "gat":
            dr[f"wl_{rname}"] = nc.dram_tensor(f"wl_{rname}", [L, 128, H * D], BF16, kind="ExternalInput")
            dr[f"wr_{rname}"] = nc.dram_tensor(f"wr_{rname}", [L, 128, H * D], BF16, kind="ExternalInput")
            dr[f"att_{rname}"] = nc.dram_tensor(f"att_{rname}", [L, 128, H * D], BF16, kind="ExternalInput")
            dr[f"gb_{rname}"] = nc.dram_tensor(f"gb_{rname}", [L, 128, D], F32, kind="ExternalInput")
        else:
            dr[f"wt_{rname}"] = nc.dram_tensor(f"wt_{rname}", [L, 128, 2 * D], BF16, kind="ExternalInput")
            dr[f"wb_{rname}"] = nc.dram_tensor(f"wb_{rname}", [L, 128, 2 * D], BF16, kind="ExternalInput")
            dr[f"cb_{rname}"] = nc.dram_tensor(f"cb_{rname}", [L, 1, 2 * D], BF16, kind="ExternalInput")
    dr["nw_w"] = nc.dram_tensor("nw_w", [L, 128, D], BF16, kind="ExternalInput")
    dr["nw_b"] = nc.dram_tensor("nw_b", [L, 128, 1], F32, kind="ExternalInput")
    dr["iota"] = nc.dram_tensor("iota", [128, 128], F32, kind="ExternalInput")
    dr["ident_f"] = nc.dram_tensor("ident_f", [128, 128], F32, kind="ExternalInput")
    dr["ident_b"] = nc.dram_tensor("ident_b", [128, 128], BF16, kind="ExternalInput")
    dr["out_my"] = nc.dram_tensor("out_my", [SHARD, D], F32, kind="ExternalOutput")
    dr["out_opp"] = nc.dram_tensor("out_opp", [SHARD, D], F32, kind="ExternalOutput")

    def ld3(pool, name, src, cols):
        t = pool.tile([128, L * cols], src.dtype, name=name, tag=name)
        nc.sync.dma_start(
            t[:].rearrange("p (l n) -> p l n", l=L),
            src[:].rearrange("l p n -> p l n"),
        )
        return t

    with tile.TileContext(nc) as tc:
        with tc.tile_pool(name="const", bufs=1) as cst, \
             tc.tile_pool(name="xwp", bufs=1) as xwp, \
             tc.tile_pool(name="accp", bufs=1) as accp, \
             tc.tile_pool(name="gth", bufs=2) as gth, \
             tc.tile_pool(name="wrk", bufs=3) as wrk, \
             tc.tile_pool(name="epi", bufs=1) as epi, \
             tc.tile_pool(name="dram", bufs=1, space="DRAM") as drm, \
             tc.tile_pool(name="pz", bufs=2, space=bass.MemorySpace.PSUM) as pzp, \
             tc.tile_pool(name="pagg", bufs=2, space=bass.MemorySpace.PSUM) as paggp, \
             tc.tile_pool(name="ps", bufs=2, space=bass.MemorySpace.PSUM) as psp:

            # ---------------- constants / inputs resident in SBUF
            xw = {}
            for ty in ("my", "opp"):
                xw[ty] = xwp.tile([128, RANKS * D], BF16, name=f"xw_{ty}_sb", tag=f"xw_{ty}_sb")
                nc.sync.dma_start(xw[ty][:], dr[f"xw_{ty}"][:])
            xres = {}
            for ty in ("my", "opp"):
                xres[ty] = xwp.tile([128, TILES * D], BF16, name=f"xres_{ty}_sb", tag=f"xres_{ty}_sb")
                nc.sync.dma_start(xres[ty][:], dr[f"xres_{ty}"][:])

            cw = {}
            for rname, kind, _, _ in RELS:
                si = cst.tile([128, EP // 16], I16, name=f"si_{rname}_sb", tag=f"si_{rname}_sb")
                nc.sync.dma_start(si[:], dr[f"si_{rname}"][:])
                di = cst.tile([128, EP // 16], I16, name=f"di_{rname}_sb", tag=f"di_{rname}_sb")
                nc.sync.dma_start(di[:], dr[f"di_{rname}"][:])
                dl = cst.tile([128, EP // 128], F32, name=f"dl_{rname}_sb", tag=f"dl_{rname}_sb")
                nc.sync.dma_start(dl[:], dr[f"dl_{rname}"][:])
                cw[rname] = {"si": si, "di": di, "dl": dl}
                if kind == "gat":
                    cw[rname]["wl"] = ld3(cst, f"wl_{rname}_sb", dr[f"wl_{rname}"], H * D)
                    cw[rname]["wr"] = ld3(cst, f"wr_{rname}_sb", dr[f"wr_{rname}"], H * D)
                    cw[rname]["att"] = ld3(cst, f"att_{rname}_sb", dr[f"att_{rname}"], H * D)
                    cw[rname]["gb"] = ld3(cst, f"gb_{rname}_sb", dr[f"gb_{rname}"], D)
                else:
                    cw[rname]["wt"] = ld3(cst, f"wt_{rname}_sb", dr[f"wt_{rname}"], 2 * D)
                    cw[rname]["wb"] = ld3(cst, f"wb_{rname}_sb", dr[f"wb_{rname}"], 2 * D)
                    cbt = cst.tile([1, L * 2 * D], BF16, name=f"cb_{rname}_sb", tag=f"cb_{rname}_sb")
                    nc.sync.dma_start(
                        cbt[:].rearrange("p (l n) -> p l n", l=L),
                        dr[f"cb_{rname}"][:].rearrange("l p n -> p l n"),
                    )
                    cw[rname]["cb"] = cbt
            nw_w = ld3(cst, "nw_w_sb", dr["nw_w"], D)
            nw_b = ld3(cst, "nw_b_sb", dr["nw_b"], 1)
            iota = cst.tile([128, 128], F32, name="iota_sb", tag="iota_sb")
            nc.sync.dma_start(iota[:], dr["iota"][:])
            ident_f = cst.tile([128, 128], F32, name="identf_sb", tag="identf_sb")
            nc.sync.dma_start(ident_f[:], dr["ident_f"][:])
            ident_b = cst.tile([128, 128], BF16, name="identb_sb", tag="identb_sb")
            nc.sync.dma_start(ident_b[:], dr["ident_b"][:])
            ones_b = cst.tile([1, 128], BF16, name="ones_sb", tag="ones_sb")
            nc.gpsimd.memset(ones_b[:], 1.0)

            # ---------------- layers
            for l in range(k_layers):
                acc_written = set()
                ACC = {}
                for ty in ("my", "opp"):
                    ACC[ty] = accp.tile([128, TILES * D], F32, name=f"acc_{ty}_{l}", tag=f"acc_{ty}")

                for rname, kind, sty, dty in rels_active:
                    cwr = cw[rname]
                    # gather chunks (feature-major, [128, EPQ] bf16)
                    xsq, xdq = [], []
                    for q in range(NCH if k_stage != "nogather" else 0):
                        xs = gth.tile([128, EPQ], BF16, name=f"xs_{rname}_{l}_{q}", tag="xs")
                        xd = gth.tile([128, EPQ], BF16, name=f"xd_{rname}_{l}_{q}", tag="xd")
                        for t_, srcw, idxt in ((xs, xw[sty], cwr["si"]), (xd, xw[dty], cwr["di"])):
                            nc.gpsimd.dma_gather(
                                out_ap=t_[:].rearrange("p (o n) -> p o n", o=1),
                                in_ap=srcw[:],
                                idxs_ap=idxt[:, q * (EPQ // 16):(q + 1) * (EPQ // 16)],
                                num_idxs=EPQ, num_idxs_reg=EPQ,
                                elem_size=128, transpose=True,
                                single_packet=False,
                                sbuf_tokens_per_rank=128,
                                sbuf_free_dim_per_rank=256,
                                sbuf_free_dim_pad_per_rank=0,
                                sbuf_byte_offset=0,
                            )
                        xsq.append(xs)
                        xdq.append(xd)

                    for t in range(TILES):
                        if k_stage == "nogather":
                            nc.vector.tensor_copy(ACC[dty][:, t * D:(t + 1) * D],
                                                  xres[dty][:, t * D:(t + 1) * D])
                            continue
                        q, tq = t // GTILES, t % GTILES
                        xs, xd = xsq[q], xdq[q]
                        if kind == "gat":
                            pagg = paggp.tile([128, H * D], F32, name=f"pagg_{rname}_{l}_{t}", tag="pagg")
                            psum_s = psp.tile([128, H], F32, name=f"psums_{rname}_{l}_{t}", tag="ps")
                        else:
                            pagg = paggp.tile([128, D], F32, name=f"pagg_{rname}_{l}_{t}", tag="pagg")
                        for b in range(Bmax):
                            off = (tq * Bmax + b) * 128
                            eb = t * Bmax + b
                            if k_stage in ("full",):
                                oh = wrk.tile([128, 128], BF16, name=f"oh_{rname}_{l}_{t}_{b}", tag="oh")
                                nc.gpsimd.tensor_scalar(
                                    oh[:], iota[:], cwr["dl"][:, eb:eb + 1], None,
                                    op0=OP.is_equal)
                            first, last = (b == 0), (b == Bmax - 1)
                            if kind == "gat" and k_stage == "gather":
                                pass
                            elif kind == "gat":
                                psz = pzp.tile([128, H * D], F32, name=f"psz_{l}_{t}_{b}", tag="pz")
                                nc.tensor.matmul(psz[:], xs[:, off:off + 128],
                                                 cwr["wl"][:, l * H * D:(l + 1) * H * D],
                                                 start=True, stop=False)
                                xlb = wrk.tile([128, H * D], BF16, name=f"xlb_{l}_{t}_{b}", tag="xlb")
                                nc.vector.tensor_copy(xlb[:], psz[:])
                                nc.tensor.matmul(psz[:], xd[:, off:off + 128],
                                                 cwr["wr"][:, l * H * D:(l + 1) * H * D],
                                                 start=False, stop=True)
                                if k_stage == "mm":
                                    if b == 0:
                                        nc.vector.tensor_copy(ACC[dty][:, t * D:(t + 1) * D], psz[:, 0:D])
                                    continue
                                z = wrk.tile([128, H * D], BF16, name=f"z_{l}_{t}_{b}", tag="z")
                                nc.scalar.activation(z[:], psz[:], AF.Prelu, alpha=0.2)
                                sc = wrk.tile([128, H], F32, name=f"sc_{l}_{t}_{b}", tag="sc")
                                scp = wrk.tile([128, H * D], BF16, name=f"scp_{l}_{t}_{b}", tag="scp")
                                nc.vector.tensor_tensor(
                                    scp[:], z[:],
                                    cwr["att"][:, l * H * D:(l + 1) * H * D], op=OP.mult)
                                nc.vector.tensor_reduce(
                                    sc[:], scp[:].rearrange("p (h f) -> p h f", f=D),
                                    axis=mybir.AxisListType.X, op=OP.add)
                                es = wrk.tile([128, H], F32, name=f"es_{l}_{t}_{b}", tag="es")
                                nc.scalar.activation(es[:], sc[:], AF.Exp)
                                es_b = wrk.tile([128, H], BF16, name=f"esb_{l}_{t}_{b}", tag="esb")
                                nc.scalar.copy(es_b[:], es[:])
                                if k_stage == "act":
                                    if b == 0:
                                        nc.vector.tensor_copy(ACC[dty][:, t * D:(t + 1) * D], z[:, 0:D])
                                    continue
                                xlw = wrk.tile([128, H * D], BF16, name=f"xlw_{l}_{t}_{b}", tag="xlw")
                                for h in range(H):
                                    eng = nc.vector if h < 2 else nc.gpsimd
                                    eng.tensor_scalar(
                                        xlw[:, h * D:(h + 1) * D], xlb[:, h * D:(h + 1) * D],
                                        es[:, h:h + 1], None, op0=OP.mult)
                                if k_stage == "xlw":
                                    if b == 0:
                                        nc.vector.tensor_copy(ACC[dty][:, t * D:(t + 1) * D], xlw[:, 0:D])
                                    continue
                                nc.tensor.matmul(pagg[:], oh[:], xlw[:], start=first, stop=last)
                                nc.tensor.matmul(psum_s[:], oh[:], es_b[:], start=first, stop=last)
                            elif k_stage == "gather":
                                pass
                            else:
                                psm = pzp.tile([128, 2 * D], F32, name=f"psm_{l}_{t}_{b}", tag="pz")
                                nc.tensor.matmul(psm[:], xd[:, off:off + 128],
                                                 cwr["wt"][:, l * 2 * D:(l + 1) * 2 * D],
                                                 start=True, stop=False)
                                nc.tensor.matmul(psm[:], xs[:, off:off + 128],
                                                 cwr["wb"][:, l * 2 * D:(l + 1) * 2 * D],
                                                 start=False, stop=False)
                                nc.tensor.matmul(psm[:], ones_b[:],
                                                 cwr["cb"][:, l * 2 * D:(l + 1) * 2 * D],
                                                 start=False, stop=True)
                                if k_stage == "mm":
                                    if b == 0:
                                        nc.vector.tensor_copy(ACC[dty][:, t * D:(t + 1) * D], psm[:, 0:D])
                                    continue
                                sgx = wrk.tile([128, D], F32, name=f"sgx_{l}_{t}_{b}", tag="sgx")
                                nc.scalar.activation(sgx[:], psm[:, 0:D], AF.Exp, scale=-1.0)
                                sgd = wrk.tile([128, D], F32, name=f"sgd_{l}_{t}_{b}", tag="sgd")
                                nc.vector.tensor_scalar(sgd[:], sgx[:], 1.0, None, op0=OP.add)
                                sg = wrk.tile([128, D], F32, name=f"sg_{l}_{t}_{b}", tag="sg")
                                nc.vector.reciprocal(sg[:], sgd[:])
                                spx = wrk.tile([128, D], F32, name=f"spx_{l}_{t}_{b}", tag="spx")
                                nc.scalar.activation(spx[:], psm[:, D:2 * D], AF.Exp)
                                sp = wrk.tile([128, D], F32, name=f"sp_{l}_{t}_{b}", tag="sp")
                                nc.scalar.activation(sp[:], spx[:], AF.Ln, bias=1.0)
                                m = wrk.tile([128, D], BF16, name=f"m_{l}_{t}_{b}", tag="m")
                                nc.vector.tensor_tensor(m[:], sg[:], sp[:], op=OP.mult)
                                if k_stage == "act":
                                    if b == 0:
                                        nc.vector.tensor_copy(ACC[dty][:, t * D:(t + 1) * D], m[:])
                                    continue
                                nc.tensor.matmul(pagg[:], oh[:], m[:], start=first, stop=last)
                        # -------- tile epilogue
                        if k_stage == "gather":
                            nc.vector.tensor_copy(ACC[dty][:, t * D:(t + 1) * D],
                                                  xs[:, (tq * Bmax) * 128:(tq * Bmax) * 128 + D])
                            continue
                        if k_stage in ("mm", "act", "xlw"):
                            continue
                        asl = ACC[dty][:, t * D:(t + 1) * D]
                        if kind == "cg":
                            if (dty, t) in acc_written:
                                nc.vector.tensor_tensor(asl, asl, pagg[:], op=OP.add)
                                nc.vector.tensor_tensor(
                                    asl, asl, xres[dty][:, t * D:(t + 1) * D], op=OP.add)
                            else:
                                nc.vector.scalar_tensor_tensor(
                                    asl, pagg[:], 1.0, xres[dty][:, t * D:(t + 1) * D],
                                    op0=OP.mult, op1=OP.add)
                            acc_written.add((dty, t))
                        else:
                            sden = wrk.tile([128, H], F32, name=f"sden_{l}_{t}", tag="sden")
                            nc.vector.tensor_scalar(sden[:], psum_s[:], 1e-16, 4.0,
                                                    op0=OP.add, op1=OP.mult)
                            inv4 = wrk.tile([128, H], F32, name=f"inv4_{l}_{t}", tag="inv4")
                            nc.vector.reciprocal(inv4[:], sden[:])
                            gt = wrk.tile([128, D], F32, name=f"gt_{l}_{t}", tag="gt")
                            nc.vector.scalar_tensor_tensor(
                                gt[:], pagg[:, 0:D], inv4[:, 0:1],
                                cwr["gb"][:, l * D:(l + 1) * D], op0=OP.mult, op1=OP.add)
                            for h in range(1, H):
                                nc.vector.scalar_tensor_tensor(
                                    gt[:], pagg[:, h * D:(h + 1) * D], inv4[:, h:h + 1],
                                    gt[:], op0=OP.mult, op1=OP.add)
                            if (dty, t) in acc_written:
                                nc.vector.tensor_tensor(asl, asl, gt[:], op=OP.add)
                            else:
                                nc.vector.tensor_copy(asl, gt[:])
                            acc_written.add((dty, t))

                # ---------------- layer epilogue: nodewise linear + layout
                last_layer = (l == k_layers - 1)
                if not last_layer:
                    ag_in = drm.tile([128, 2 * TILES * D], BF16, name=f"agin_{l}", tag="agin")
                    ag_out = drm.tile([CORES * 128, 2 * TILES * D], BF16,
                                      name=f"agout_{l}", tag="agout", addr_space="Shared")
                for tyi, ty in enumerate(("my", "opp")):
                    if ty not in {r[3] for r in rels_active}:
                        continue
                    if not k_epi:
                        if last_layer:
                            for t in range(TILES):
                                osb0 = wrk.tile([128, 128], F32, name=f"osb0_{ty}_{l}_{t}", tag="osb")
                                nc.vector.tensor_copy(osb0[:], ACC[ty][:, t * D:(t + 1) * D])
                                nc.sync.dma_start(dr[f"out_{ty}"][t * 128:(t + 1) * 128, :], osb0[:])
                        continue
                    accT = epi.tile([128, TILES * D], BF16, name=f"accT_{ty}_{l}", tag="accT")
                    for t in range(TILES):
                        ptr = psp.tile([128, 128], F32, name=f"ptr_{ty}_{l}_{t}", tag="ps")
                        nc.tensor.transpose(ptr[:], ACC[ty][:, t * D:(t + 1) * D], ident_f[:])
                        nc.scalar.copy(accT[:, t * D:(t + 1) * D], ptr[:])
                    xnT = epi.tile([128, TILES * D], BF16 if not last_layer else F32,
                                   name=f"xnT_{ty}_{l}", tag="xnT")
                    for k in range(TILES * D // 512):
                        pnw = paggp.tile([128, 512], F32, name=f"pnw_{ty}_{l}_{k}", tag="pagg")
                        nc.tensor.matmul(pnw[:], nw_w[:, l * D:(l + 1) * D],
                                         accT[:, k * 512:(k + 1) * 512],
                                         start=True, stop=True)
                        nc.scalar.activation(xnT[:, k * 512:(k + 1) * 512], pnw[:],
                                             AF.Identity, bias=nw_b[:, l:l + 1])
                    # back to dst-major
                    for t in range(TILES):
                        if not last_layer:
                            ptr2 = psp.tile([128, 128], BF16, name=f"ptr2_{ty}_{l}_{t}", tag="ps")
                            nc.tensor.transpose(ptr2[:], xnT[:, t * D:(t + 1) * D], ident_b[:])
                            nc.vector.tensor_copy(xres[ty][:, t * D:(t + 1) * D], ptr2[:])
                        else:
                            ptr2 = psp.tile([128, 128], F32, name=f"ptr2_{ty}_{l}_{t}", tag="ps")
                            nc.tensor.transpose(ptr2[:], xnT[:, t * D:(t + 1) * D], ident_f[:])
                            osb = wrk.tile([128, 128], F32, name=f"osb_{ty}_{l}_{t}", tag="osb")
                            nc.vector.tensor_copy(osb[:], ptr2[:])
                            nc.sync.dma_start(dr[f"out_{ty}"][t * 128:(t + 1) * 128, :], osb[:])
                    if not last_layer:
                        nc.sync.dma_start(
                            ag_in[:, tyi * TILES * D:(tyi + 1) * TILES * D], xres[ty][:])
                if not last_layer and k_ag:
                    nc.gpsimd.collective_compute(
                        "AllGather", mybir.AluOpType.bypass,
                        replica_groups=[list(range(CORES))],
                        ins=[ag_in.opt()], outs=[ag_out.opt()],
                    )
                    for tyi, ty in enumerate(("my", "opp")):
                        nc.sync.dma_start(
                            xw[ty][:].rearrange("p (c j) -> p c j", c=CORES),
                            ag_out[:, tyi * TILES * D:(tyi + 1) * TILES * D]
                            .rearrange("(c p) j -> p c j", p=128),
                        )

    nc.compile()
    return nc


_prog_cache = {}


def _get_program(Bmax):
    if Bmax not in _prog_cache:
        _prog_cache[Bmax] = _build_program(Bmax)
    return _prog_cache[Bmax]


# ------------------------------------------------------------------- kernel

def kernel(**inputs):
    global LAST_EXEC_NS
    from concourse.bass_utils import run_bass_kernel_spmd

    f32 = lambda k: np.asarray(inputs[k], np.float32)
    x_my, x_opp = f32("x_my"), f32("x_opp")

    # edges
    eprep = {}
    Bmax = 1
    for rname, key in (("loses", "ei_loses"), ("beats", "ei_beats"),
                       ("rev_beats", "ei_rev_beats"), ("rev_loses", "ei_rev_loses")):
        percore, mc = _prep_edges(np.asarray(inputs[key]))
        eprep[rname] = percore
        Bmax = max(Bmax, -(-mc // 128))
    packed = {r: _pack_edges(eprep[r], Bmax) for r in eprep}

    nc = _get_program(Bmax)

    # shared (per-core identical) tensors
    shared = {}
    shared["xw_my"] = _wrap_nodes(x_my)
    shared["xw_opp"] = _wrap_nodes(x_opp)
    for rname, kind, _, _ in RELS:
        tag = {"loses": "cg_lose", "beats": "gat_beats",
               "rev_beats": "cg_rev", "rev_loses": "gat_rev"}[rname]
        if kind == "gat":
            shared[f"wl_{rname}"] = np.ascontiguousarray(f32(f"{tag}_Wl")).astype(BF)
            shared[f"wr_{rname}"] = np.ascontiguousarray(f32(f"{tag}_Wr")).astype(BF)
            att = f32(f"{tag}_att")  # [L, H, D]
            shared[f"att_{rname}"] = np.stack(
                [_rep(att[l].reshape(-1)) for l in range(L)]).astype(BF)
            b = f32(f"{tag}_b")  # [L, D]
            shared[f"gb_{rname}"] = np.stack([_rep(b[l]) for l in range(L)])
        else:
            wf, ws = f32(f"{tag}_Wf"), f32(f"{tag}_Ws")  # [L, 2D, D]
            shared[f"wt_{rname}"] = np.ascontiguousarray(
                np.concatenate([wf[:, :D, :], ws[:, :D, :]], axis=2)).astype(BF)
            shared[f"wb_{rname}"] = np.ascontiguousarray(
                np.concatenate([wf[:, D:, :], ws[:, D:, :]], axis=2)).astype(BF)
            bfv, bsv = f32(f"{tag}_bf"), f32(f"{tag}_bs")  # [L, D]
            shared[f"cb_{rname}"] = np.ascontiguousarray(
                np.concatenate([bfv, bsv], axis=1).reshape(L, 1, 2 * D)).astype(BF)
    shared["nw_w"] = np.ascontiguousarray(f32("nw_W")).astype(BF)
    shared["nw_b"] = np.ascontiguousarray(f32("nw_b").reshape(L, 128, 1))
    shared["iota"] = np.tile(np.arange(128, dtype=np.float32), (128, 1))
    shared["ident_f"] = np.eye(128, dtype=np.float32)
    shared["ident_b"] = np.eye(128).astype(BF)

    in_maps = []
    for c in range(CORES):
        m = dict(shared)
        m["xres_my"] = _dst_major_slice(x_my, c)
        m["xres_opp"] = _dst_major_slice(x_opp, c)
        for rname in packed:
            s_a, d_a, l_a = packed[rname][c]
            m[f"si_{rname}"] = _idx_dev(s_a)
            m[f"di_{rname}"] = _idx_dev(d_a)
            m[f"dl_{rname}"] = _loc_dev(l_a)
        in_maps.append(m)

    trace = os.environ.get("KERNEL_PROFILE", "0") == "1"
    res = run_bass_kernel_spmd(nc, in_maps, core_ids=list(range(CORES)),
                               trace=trace, trace_cores=[0] if trace else None)
    LAST_EXEC_NS = res.exec_time_ns

    out_my = np.concatenate([res.results[c]["out_my"] for c in range(CORES)])[:N]
    out_opp = np.concatenate([res.results[c]["out_opp"] for c in range(CORES)])[:N]
    return out_my, out_opp



# revision 3
# speedup vs baseline: 2.2313x; 2.2313x over previous
"""Trainium2 Bass kernel for the 2-layer heterogeneous GNN (GATv2 + CGConv).

Sharding: destination nodes (both node types) are split into 8 contiguous
ranges of 2560 (N padded 20000 -> 20480); each core owns the edges that
target its range, for all 4 relations.  Node features are replicated
(SBUF-resident, bf16, node-wrapped layout) so per-edge source gathers are
SBUF->SBUF dma_gather ops (feature-major output); destination-side
per-edge values come from one-hot selector matmuls on the PE.  The
one-hot matrices (static, from the edge lists) are precomputed on the
host and streamed from HBM.  The inter-layer halo exchange is a single
AllGather of the updated 2560-row slices.
"""

import os
import numpy as np
import ml_dtypes

BF = ml_dtypes.bfloat16

N = 20000
D = 128
H = 4
L = 2
E = 80000
CORES = 8
NPAD = 20480
SHARD = 2560
TILES = 20           # dst tiles of 128 per core
RANKS = NPAD // 128  # 160
PAD_NODE = 20000     # zero-feature padding node (valid gather target)
GTILES = 4           # dst tiles per gather chunk

LAST_EXEC_NS = None

# relation table: (name, kind, src_type, dst_type); cg before gat per dst type
RELS = [
    ("loses", "cg", "my", "opp"),
    ("beats", "gat", "my", "opp"),
    ("rev_beats", "cg", "opp", "my"),
    ("rev_loses", "gat", "opp", "my"),
]


# ----------------------------------------------------------------- host prep

def _wrap_nodes(x):
    """[N,128] f32 -> node-wrapped [128, RANKS*128] bf16 (node n at
    partition n%128, cols (n//128)*128 : +128)."""
    xp = np.zeros((NPAD, D), np.float32)
    xp[:N] = x
    return np.ascontiguousarray(
        xp.reshape(RANKS, 128, D).transpose(1, 0, 2).reshape(128, RANKS * D)
    ).astype(BF)


def _dst_major_slice(x, c):
    """core c's own dst slice, dst-major [128, TILES*128] bf16."""
    xp = np.zeros((NPAD, D), np.float32)
    xp[:N] = x
    sl = xp[c * SHARD:(c + 1) * SHARD]
    return np.ascontiguousarray(
        sl.reshape(TILES, 128, D).transpose(1, 0, 2).reshape(128, TILES * D)
    ).astype(BF)


def _feat_major_slice(x, c):
    """core c's own dst slice, feature-major [128, TILES*128] bf16
    (col t*128+j = node c*2560+t*128+j)."""
    xp = np.zeros((NPAD, D), np.float32)
    xp[:N] = x
    sl = xp[c * SHARD:(c + 1) * SHARD]  # [2560, D]
    return np.ascontiguousarray(
        sl.reshape(TILES, 128, D).transpose(2, 0, 1).reshape(D, TILES * 128)
    ).astype(BF)


def _prep_edges(ei):
    """bucket edges by (core, dst tile); returns per-core lists + max count."""
    src = np.asarray(ei[0]).astype(np.int64)
    dst = np.asarray(ei[1]).astype(np.int64)
    percore = []
    maxcnt = 1
    for c in range(CORES):
        m = (dst >= c * SHARD) & (dst < (c + 1) * SHARD)
        s, d = src[m], dst[m]
        dl = d - c * SHARD
        tid = dl // 128
        buckets = [np.nonzero(tid == t)[0] for t in range(TILES)]
        for b in buckets:
            maxcnt = max(maxcnt, len(b))
        percore.append((s, dl, buckets))
    return percore, maxcnt


def _pack_edges(percore, Bmax):
    """-> per-core (src_ids [EP], dloc [EP]); EP = TILES*Bmax*128, pad=-1."""
    out = []
    for (s, dl, buckets) in percore:
        src_a = np.full((TILES, Bmax * 128), PAD_NODE, np.int64)
        loc_a = np.full((TILES, Bmax * 128), -1, np.int64)
        for t, b in enumerate(buckets):
            n = len(b)
            src_a[t, :n] = s[b]
            loc_a[t, :n] = dl[b] % 128
        out.append((src_a.reshape(-1), loc_a.reshape(-1)))
    return out


def _onehots(loc, Bmax):
    """loc [EP] (-1 = pad) -> (oh_e [128, NB*128], oh_d [128, NB*128]) bf16.

    oh_e block gb: [j=edge-in-block, d=dst-local]; oh_d block = transpose."""
    NB = TILES * Bmax
    EP = NB * 128
    M = np.zeros((EP, 128), np.float32)
    valid = loc >= 0
    M[np.nonzero(valid)[0], loc[valid]] = 1.0
    Mb = M.reshape(NB, 128, 128)
    oh_e = np.ascontiguousarray(Mb.transpose(1, 0, 2).reshape(128, NB * 128))
    oh_d = np.ascontiguousarray(Mb.transpose(2, 0, 1).reshape(128, NB * 128))
    return oh_e.astype(BF), oh_d.astype(BF)


def _idx_dev(a):
    """[EP] int -> [128, EP//16] int16 (16-partition wrap, replicated 8x)."""
    x = a.astype(np.int16).reshape(-1, 16).T
    return np.ascontiguousarray(np.tile(x, (8, 1)))


def _rep(v, rows=128):
    return np.ascontiguousarray(
        np.tile(np.asarray(v, np.float32).reshape(1, -1), (rows, 1)))


# ------------------------------------------------------------- program build

def _build_program(Bmax):
    import concourse.bass as bass
    import concourse.bacc as bacc
    import concourse.mybir as mybir
    import concourse.tile as tile
    from concourse.hw_specs import get_activation_tables

    F32, BF16, I16 = mybir.dt.float32, mybir.dt.bfloat16, mybir.dt.int16
    AF = mybir.ActivationFunctionType
    OP = mybir.AluOpType

    NB = TILES * Bmax
    EP = NB * 128
    EPQ = GTILES * Bmax * 128       # idxs per gather chunk
    NCH = TILES // GTILES           # gather chunks per relation
    CB = GTILES * Bmax              # blocks per chunk

    k_layers = int(os.environ.get("K_LAYERS", str(L)))
    k_rels = os.environ.get("K_RELS", "")
    rels_active = [r for r in RELS if (not k_rels or r[0] in k_rels.split(","))]

    nc = bacc.Bacc("TRN2", target_bir_lowering=False, debug=False,
                   num_devices=CORES)

    dr = {}
    dr["xw_my"] = nc.dram_tensor("xw_my", [128, RANKS * D], BF16, kind="ExternalInput")
    dr["xw_opp"] = nc.dram_tensor("xw_opp", [128, RANKS * D], BF16, kind="ExternalInput")
    for ty in ("my", "opp"):
        dr[f"xres_{ty}"] = nc.dram_tensor(f"xres_{ty}", [128, TILES * D], BF16, kind="ExternalInput")
        dr[f"xfm_{ty}"] = nc.dram_tensor(f"xfm_{ty}", [128, TILES * 128], BF16, kind="ExternalInput")
    for rname, kind, _, _ in RELS:
        dr[f"si_{rname}"] = nc.dram_tensor(f"si_{rname}", [128, EP // 16], I16, kind="ExternalInput")
        dr[f"ohe_{rname}"] = nc.dram_tensor(f"ohe_{rname}", [128, NB * 128], BF16, kind="ExternalInput")
        dr[f"ohd_{rname}"] = nc.dram_tensor(f"ohd_{rname}", [128, NB * 128], BF16, kind="ExternalInput")
        if kind == "gat":
            dr[f"wl_{rname}"] = nc.dram_tensor(f"wl_{rname}", [L, 128, H * D], BF16, kind="ExternalInput")
            dr[f"wr_{rname}"] = nc.dram_tensor(f"wr_{rname}", [L, 128, H * D], BF16, kind="ExternalInput")
            dr[f"att_{rname}"] = nc.dram_tensor(f"att_{rname}", [L, 128, H * D], BF16, kind="ExternalInput")
            dr[f"gb_{rname}"] = nc.dram_tensor(f"gb_{rname}", [L, 128, D], F32, kind="ExternalInput")
        else:
            dr[f"wt_{rname}"] = nc.dram_tensor(f"wt_{rname}", [L, 128, 2 * D], BF16, kind="ExternalInput")
            dr[f"wb_{rname}"] = nc.dram_tensor(f"wb_{rname}", [L, 128, 2 * D], BF16, kind="ExternalInput")
            dr[f"cb_{rname}"] = nc.dram_tensor(f"cb_{rname}", [L, 1, 2 * D], BF16, kind="ExternalInput")
    dr["nw_w"] = nc.dram_tensor("nw_w", [L, 128, D], BF16, kind="ExternalInput")
    dr["nw_b"] = nc.dram_tensor("nw_b", [L, 128, 1], F32, kind="ExternalInput")
    dr["ident_f"] = nc.dram_tensor("ident_f", [128, 128], F32, kind="ExternalInput")
    dr["ident_b"] = nc.dram_tensor("ident_b", [128, 128], BF16, kind="ExternalInput")
    dr["out_my"] = nc.dram_tensor("out_my", [128, TILES * D], F32, kind="ExternalOutput")
    dr["out_opp"] = nc.dram_tensor("out_opp", [128, TILES * D], F32, kind="ExternalOutput")

    def ld3(pool, name, src, cols, dt=None):
        t = pool.tile([128, L * cols], dt or src.dtype, name=name, tag=name)
        nc.sync.dma_start(
            t[:].rearrange("p (l n) -> p l n", l=L),
            src[:].rearrange("l p n -> p l n"),
        )
        return t

    with tile.TileContext(nc) as tc:
        with tc.tile_pool(name="const", bufs=1) as cst, \
             tc.tile_pool(name="xwp", bufs=1) as xwp, \
             tc.tile_pool(name="accp", bufs=1) as accp, \
             tc.tile_pool(name="gth", bufs=2) as gth, \
             tc.tile_pool(name="ohp", bufs=2) as ohp, \
             tc.tile_pool(name="wrk", bufs=3) as wrk, \
             tc.tile_pool(name="til", bufs=2) as til, \
             tc.tile_pool(name="epi", bufs=1) as epi, \
             tc.tile_pool(name="dram", bufs=1, space="DRAM") as drm, \
             tc.tile_pool(name="pz", bufs=4, space=bass.MemorySpace.PSUM) as pzp, \
             tc.tile_pool(name="pagg", bufs=2, space=bass.MemorySpace.PSUM) as paggp, \
             tc.tile_pool(name="pden", bufs=2, space=bass.MemorySpace.PSUM) as pdenp:

            # one activation table serves Exp/Ln/Prelu/Copy/Identity
            tabs = list(get_activation_tables(nc.m.arch).items())
            need = {AF.Exp, AF.Ln, AF.Prelu, AF.Copy, AF.Identity}
            set_id = next(i for i, (_, fns) in enumerate(tabs) if need <= fns)
            nc.scalar.add_instruction(mybir.InstLoadActFuncSet(
                name=nc.get_next_instruction_name(), ins=[], outs=[],
                act_func_set_id=set_id))

            # ---------------- persistent SBUF state
            xw = {}
            for ty in ("my", "opp"):
                xw[ty] = xwp.tile([128, RANKS * D], BF16, name=f"xw_{ty}_sb", tag=f"xw_{ty}_sb")
                nc.sync.dma_start(xw[ty][:], dr[f"xw_{ty}"][:])
            xres, xfm = {}, {}
            for ty in ("my", "opp"):
                xres[ty] = xwp.tile([128, TILES * D], BF16, name=f"xres_{ty}_sb", tag=f"xres_{ty}_sb")
                nc.sync.dma_start(xres[ty][:], dr[f"xres_{ty}"][:])
                xfm[ty] = xwp.tile([128, TILES * 128], BF16, name=f"xfm_{ty}_sb", tag=f"xfm_{ty}_sb")
                nc.sync.dma_start(xfm[ty][:], dr[f"xfm_{ty}"][:])

            cw = {}
            for rname, kind, _, _ in RELS:
                si = cst.tile([128, EP // 16], I16, name=f"si_{rname}_sb", tag=f"si_{rname}_sb")
                nc.sync.dma_start(si[:], dr[f"si_{rname}"][:])
                cw[rname] = {"si": si}
                if kind == "gat":
                    cw[rname]["wl"] = ld3(cst, f"wl_{rname}_sb", dr[f"wl_{rname}"], H * D)
                    cw[rname]["wr"] = ld3(cst, f"wr_{rname}_sb", dr[f"wr_{rname}"], H * D)
                    cw[rname]["att"] = ld3(cst, f"att_{rname}_sb", dr[f"att_{rname}"], H * D)
                    cw[rname]["gb"] = ld3(cst, f"gb_{rname}_sb", dr[f"gb_{rname}"], D)
                else:
                    cw[rname]["wt"] = ld3(cst, f"wt_{rname}_sb", dr[f"wt_{rname}"], 2 * D)
                    cw[rname]["wb"] = ld3(cst, f"wb_{rname}_sb", dr[f"wb_{rname}"], 2 * D)
                    cbt = cst.tile([1, L * 2 * D], BF16, name=f"cb_{rname}_sb", tag=f"cb_{rname}_sb")
                    nc.sync.dma_start(
                        cbt[:].rearrange("p (l n) -> p l n", l=L),
                        dr[f"cb_{rname}"][:].rearrange("l p n -> p l n"),
                    )
                    cw[rname]["cb"] = cbt
            nw_w = ld3(cst, "nw_w_sb", dr["nw_w"], D)
            nw_b = ld3(cst, "nw_b_sb", dr["nw_b"], 1)
            ident_f = cst.tile([128, 128], F32, name="identf_sb", tag="identf_sb")
            nc.sync.dma_start(ident_f[:], dr["ident_f"][:])
            ident_b = cst.tile([128, 128], BF16, name="identb_sb", tag="identb_sb")
            nc.sync.dma_start(ident_b[:], dr["ident_b"][:])
            ones_b = cst.tile([1, 128], BF16, name="ones_sb", tag="ones_sb")
            nc.gpsimd.memset(ones_b[:], 1.0)

            # ---------------- layers
            for l in range(k_layers):
                ACC = {}
                for ty in ("my", "opp"):
                    ACC[ty] = accp.tile([128, TILES * D], BF16, name=f"acc_{ty}_{l}", tag=f"acc_{ty}")

                for rname, kind, sty, dty in rels_active:
                    cwr = cw[rname]

                    # gather chunks (feature-major [128 f, EPQ e] bf16)
                    xsq = []
                    ohq = []
                    for q in range(NCH):
                        xs = gth.tile([128, EPQ], BF16, name=f"xs_{rname}_{l}_{q}", tag="xs")
                        nc.gpsimd.dma_gather(
                            out_ap=xs[:].rearrange("p (o n) -> p o n", o=1),
                            in_ap=xw[sty][:],
                            idxs_ap=cwr["si"][:, q * (EPQ // 16):(q + 1) * (EPQ // 16)],
                            num_idxs=EPQ, num_idxs_reg=EPQ,
                            elem_size=128, transpose=True,
                            single_packet=False,
                            sbuf_tokens_per_rank=128,
                            sbuf_free_dim_per_rank=256,
                            sbuf_free_dim_pad_per_rank=0,
                            sbuf_byte_offset=0,
                        )
                        xsq.append(xs)
                        ohe = ohp.tile([128, CB * 128], BF16, name=f"ohe_{rname}_{l}_{q}", tag="ohe")
                        nc.sync.dma_start(ohe[:], dr[f"ohe_{rname}"][:, q * CB * 128:(q + 1) * CB * 128])
                        ohd = ohp.tile([128, CB * 128], BF16, name=f"ohd_{rname}_{l}_{q}", tag="ohd")
                        nc.scalar.dma_start(ohd[:], dr[f"ohd_{rname}"][:, q * CB * 128:(q + 1) * CB * 128])
                        ohq.append((ohe, ohd))

                    for t in range(TILES):
                        q, tq = t // GTILES, t % GTILES
                        xs = xsq[q]
                        ohe_c, ohd_c = ohq[q]

                        # ---- per-tile dst-side precompute
                        if kind == "gat":
                            pxr = pzp.tile([128, H * D], F32, name=f"pxr_{rname}_{l}_{t}", tag="pz")
                            nc.tensor.matmul(pxr[:], xfm[dty][:, t * 128:(t + 1) * 128],
                                             cwr["wr"][:, l * H * D:(l + 1) * H * D],
                                             start=True, stop=True)
                            xr_sb = til.tile([128, H * D], BF16, name=f"xrsb_{rname}_{l}_{t}", tag="xr_sb")
                            nc.scalar.copy(xr_sb[:], pxr[:])
                            pagg = paggp.tile([128, H * D], F32, name=f"pagg_{rname}_{l}_{t}", tag="pagg")
                            pden = pdenp.tile([128, H], F32, name=f"pden_{rname}_{l}_{t}", tag="pden")
                        else:
                            pud = pzp.tile([128, 2 * D], F32, name=f"pud_{rname}_{l}_{t}", tag="pz")
                            nc.tensor.matmul(pud[:], xfm[dty][:, t * 128:(t + 1) * 128],
                                             cwr["wt"][:, l * 2 * D:(l + 1) * 2 * D],
                                             start=True, stop=False)
                            nc.tensor.matmul(pud[:], ones_b[:],
                                             cwr["cb"][:, l * 2 * D:(l + 1) * 2 * D],
                                             start=False, stop=True)
                            ud_sb = til.tile([128, 2 * D], BF16, name=f"udsb_{rname}_{l}_{t}", tag="ud_sb")
                            nc.scalar.copy(ud_sb[:], pud[:])
                            pagg = paggp.tile([128, D], F32, name=f"pagg_{rname}_{l}_{t}", tag="pagg")

                        for b in range(Bmax):
                            off = (tq * Bmax + b) * 128
                            xs_fm = xs[:, off:off + 128]
                            ohe = ohe_c[:, off:off + 128]
                            ohd = ohd_c[:, off:off + 128]
                            first, last = (b == 0), (b == Bmax - 1)

                            if kind == "gat":
                                # psz_l: Wl part only (for alpha-weighted aggregation)
                                psz_l = pzp.tile([128, H * D], F32, name=f"pszl_{l}_{t}_{b}", tag="pz")
                                nc.tensor.matmul(psz_l[:], xs_fm,
                                                 cwr["wl"][:, l * H * D:(l + 1) * H * D],
                                                 start=True, stop=True)
                                # psz_f: Wl + Wr[dst] (for the score)
                                psz_f = pzp.tile([128, H * D], F32, name=f"pszf_{l}_{t}_{b}", tag="pz")
                                nc.tensor.matmul(psz_f[:], ohd, xr_sb[:],
                                                 start=True, stop=False)
                                nc.tensor.matmul(psz_f[:], xs_fm,
                                                 cwr["wl"][:, l * H * D:(l + 1) * H * D],
                                                 start=False, stop=True)
                                z = wrk.tile([128, H * D], BF16, name=f"z_{l}_{t}_{b}", tag="z")
                                nc.scalar.activation(z[:], psz_f[:], AF.Prelu, alpha=0.2)
                                scp = wrk.tile([128, H * D], BF16, name=f"scp_{l}_{t}_{b}", tag="scp")
                                nc.gpsimd.tensor_tensor(
                                    scp[:], z[:],
                                    cwr["att"][:, l * H * D:(l + 1) * H * D], op=OP.mult)
                                sc = wrk.tile([128, H], F32, name=f"sc_{l}_{t}_{b}", tag="sc")
                                nc.vector.tensor_reduce(
                                    sc[:], scp[:].rearrange("p (h f) -> p h f", f=D),
                                    axis=mybir.AxisListType.X, op=OP.add)
                                es = wrk.tile([128, H], BF16, name=f"es_{l}_{t}_{b}", tag="es")
                                nc.scalar.activation(es[:], sc[:], AF.Exp)
                                # xlw = xl * es[h]  (broadcast along feature dim)
                                xlw = wrk.tile([128, H * D], BF16, name=f"xlw_{l}_{t}_{b}", tag="xlw")
                                nc.vector.tensor_tensor(
                                    xlw[:].rearrange("p (h f) -> p h f", f=D),
                                    psz_l[:].rearrange("p (h f) -> p h f", f=D),
                                    es[:].unsqueeze(2).broadcast_to((128, H, D)),
                                    op=OP.mult)
                                nc.tensor.matmul(pagg[:], ohe, xlw[:], start=first, stop=last)
                                nc.tensor.matmul(pden[:], ohe, es[:], start=first, stop=last)
                            else:
                                psm = pzp.tile([128, 2 * D], F32, name=f"psm_{l}_{t}_{b}", tag="pz")
                                nc.tensor.matmul(psm[:], ohd, ud_sb[:],
                                                 start=True, stop=False)
                                nc.tensor.matmul(psm[:], xs_fm,
                                                 cwr["wb"][:, l * 2 * D:(l + 1) * 2 * D],
                                                 start=False, stop=True)
                                # cols 0:D hold -u; cols D:2D hold v (Wf negated on host)
                                s1 = wrk.tile([128, 2 * D], F32, name=f"s1_{l}_{t}_{b}", tag="s1")
                                nc.scalar.activation(s1[:], psm[:], AF.Exp)
                                sp = wrk.tile([128, D], F32, name=f"sp_{l}_{t}_{b}", tag="sp")
                                nc.scalar.activation(sp[:], s1[:, D:2 * D], AF.Ln, bias=1.0)
                                d1 = wrk.tile([128, D], F32, name=f"d1_{l}_{t}_{b}", tag="d1")
                                nc.vector.tensor_scalar(d1[:], s1[:, 0:D], 1.0, None, op0=OP.add)
                                rsg = wrk.tile([128, D], F32, name=f"rsg_{l}_{t}_{b}", tag="rsg")
                                nc.vector.reciprocal_approx_fast(rsg[:], d1[:])
                                m = wrk.tile([128, D], BF16, name=f"m_{l}_{t}_{b}", tag="m")
                                nc.vector.tensor_tensor(m[:], rsg[:], sp[:], op=OP.mult)
                                nc.tensor.matmul(pagg[:], ohe, m[:], start=first, stop=last)

                        # -------- tile epilogue
                        asl = ACC[dty][:, t * D:(t + 1) * D]
                        if kind == "cg":
                            # ACC = cg_agg + residual  (cg runs first per type)
                            nc.vector.scalar_tensor_tensor(
                                asl, pagg[:], 1.0, xres[dty][:, t * D:(t + 1) * D],
                                op0=OP.mult, op1=OP.add)
                        else:
                            sden = til.tile([128, H], F32, name=f"sden_{rname}_{l}_{t}", tag="sden")
                            nc.vector.tensor_scalar(sden[:], pden[:], 1e-16, 4.0,
                                                    op0=OP.add, op1=OP.mult)
                            inv4 = til.tile([128, H], F32, name=f"inv4_{rname}_{l}_{t}", tag="inv4")
                            nc.vector.reciprocal_approx_fast(inv4[:], sden[:])
                            # gtmp = pagg * inv4[h] (broadcast), then sum heads
                            gtmp = til.tile([128, H * D], F32, name=f"gtmp_{rname}_{l}_{t}", tag="gtmp")
                            nc.vector.tensor_tensor(
                                gtmp[:].rearrange("p (h f) -> p h f", f=D),
                                pagg[:].rearrange("p (h f) -> p h f", f=D),
                                inv4[:].unsqueeze(2).broadcast_to((128, H, D)),
                                op=OP.mult)
                            gt = til.tile([128, D], F32, name=f"gt_{rname}_{l}_{t}", tag="gt")
                            nc.vector.tensor_reduce(
                                gt[:], gtmp[:].rearrange("p (h f) -> p f h", f=D),
                                axis=mybir.AxisListType.X, op=OP.add)
                            # ACC += gt + gb
                            gt2 = til.tile([128, D], F32, name=f"gt2_{rname}_{l}_{t}", tag="gt2")
                            nc.vector.scalar_tensor_tensor(
                                gt2[:], gt[:], 1.0, cwr["gb"][:, l * D:(l + 1) * D],
                                op0=OP.mult, op1=OP.add)
                            nc.vector.tensor_tensor(asl, asl, gt2[:], op=OP.add)

                # ---------------- layer epilogue: nodewise linear + layout
                last_layer = (l == k_layers - 1)
                if not last_layer:
                    ag_in = drm.tile([128, 2 * TILES * D], BF16, name=f"agin_{l}", tag="agin")
                    ag_out = drm.tile([CORES * 128, 2 * TILES * D], BF16,
                                      name=f"agout_{l}", tag="agout", addr_space="Shared")
                for tyi, ty in enumerate(("my", "opp")):
                    if ty not in {r[3] for r in rels_active}:
                        continue
                    accT = epi.tile([128, TILES * D], BF16, name=f"accT_{ty}_{l}", tag="accT")
                    for t in range(TILES):
                        ptr = pzp.tile([128, 128], BF16, name=f"ptr_{ty}_{l}_{t}", tag="pz")
                        nc.tensor.transpose(ptr[:], ACC[ty][:, t * D:(t + 1) * D], ident_b[:])
                        nc.scalar.copy(accT[:, t * D:(t + 1) * D], ptr[:])
                    for k in range(TILES * D // 512):
                        pnw = paggp.tile([128, 512], F32, name=f"pnw_{ty}_{l}_{k}", tag="pagg")
                        nc.tensor.matmul(pnw[:], nw_w[:, l * D:(l + 1) * D],
                                         accT[:, k * 512:(k + 1) * 512],
                                         start=True, stop=True)
                        if last_layer:
                            osb = epi.tile([128, 512], F32, name=f"osb_{ty}_{l}_{k}", tag="osb")
                            nc.scalar.activation(osb[:], pnw[:], AF.Identity,
                                                 bias=nw_b[:, l:l + 1])
                            nc.sync.dma_start(dr[f"out_{ty}"][:, k * 512:(k + 1) * 512], osb[:])
                        else:
                            nc.scalar.activation(xfm[ty][:, k * 512:(k + 1) * 512], pnw[:],
                                                 AF.Identity, bias=nw_b[:, l:l + 1])
                    if not last_layer:
                        # back to dst-major for residuals + halo exchange
                        for t in range(TILES):
                            ptr2 = pzp.tile([128, 128], BF16, name=f"ptr2_{ty}_{l}_{t}", tag="pz")
                            nc.tensor.transpose(ptr2[:], xfm[ty][:, t * D:(t + 1) * D], ident_b[:])
                            nc.vector.tensor_copy(xres[ty][:, t * D:(t + 1) * D], ptr2[:])
                        nc.sync.dma_start(
                            ag_in[:, tyi * TILES * D:(tyi + 1) * TILES * D], xres[ty][:])
                if not last_layer:
                    nc.gpsimd.collective_compute(
                        "AllGather", mybir.AluOpType.bypass,
                        replica_groups=[list(range(CORES))],
                        ins=[ag_in.opt()], outs=[ag_out.opt()],
                    )
                    for tyi, ty in enumerate(("my", "opp")):
                        nc.sync.dma_start(
                            xw[ty][:].rearrange("p (c j) -> p c j", c=CORES),
                            ag_out[:, tyi * TILES * D:(tyi + 1) * TILES * D]
                            .rearrange("(c p) j -> p c j", p=128),
                        )

    nc.compile()
    return nc


_prog_cache = {}


def _get_program(Bmax):
    if Bmax not in _prog_cache:
        _prog_cache[Bmax] = _build_program(Bmax)
    return _prog_cache[Bmax]


# ------------------------------------------------------------------- kernel

def kernel(**inputs):
    global LAST_EXEC_NS
    from concourse.bass_utils import run_bass_kernel_spmd

    f32 = lambda k: np.asarray(inputs[k], np.float32)
    x_my, x_opp = f32("x_my"), f32("x_opp")

    # edges
    eprep = {}
    Bmax = 1
    for rname, key in (("loses", "ei_loses"), ("beats", "ei_beats"),
                       ("rev_beats", "ei_rev_beats"), ("rev_loses", "ei_rev_loses")):
        percore, mc = _prep_edges(np.asarray(inputs[key]))
        eprep[rname] = percore
        Bmax = max(Bmax, -(-mc // 128))
    packed = {r: _pack_edges(eprep[r], Bmax) for r in eprep}

    nc = _get_program(Bmax)

    # shared (per-core identical) tensors
    shared = {}
    shared["xw_my"] = _wrap_nodes(x_my)
    shared["xw_opp"] = _wrap_nodes(x_opp)
    for rname, kind, _, _ in RELS:
        tag = {"loses": "cg_lose", "beats": "gat_beats",
               "rev_beats": "cg_rev", "rev_loses": "gat_rev"}[rname]
        if kind == "gat":
            shared[f"wl_{rname}"] = np.ascontiguousarray(f32(f"{tag}_Wl")).astype(BF)
            shared[f"wr_{rname}"] = np.ascontiguousarray(f32(f"{tag}_Wr")).astype(BF)
            att = f32(f"{tag}_att")  # [L, H, D]
            shared[f"att_{rname}"] = np.stack(
                [_rep(att[l].reshape(-1)) for l in range(L)]).astype(BF)
            b = f32(f"{tag}_b")  # [L, D]
            shared[f"gb_{rname}"] = np.stack([_rep(b[l]) for l in range(L)])
        else:
            # f-gate (Wf) negated so psm[:, :D] = -u and sigmoid(u) = 1/(1+e^{psm0})
            wf, ws = -f32(f"{tag}_Wf"), f32(f"{tag}_Ws")  # [L, 2D, D]
            shared[f"wt_{rname}"] = np.ascontiguousarray(
                np.concatenate([wf[:, :D, :], ws[:, :D, :]], axis=2)).astype(BF)
            shared[f"wb_{rname}"] = np.ascontiguousarray(
                np.concatenate([wf[:, D:, :], ws[:, D:, :]], axis=2)).astype(BF)
            bfv, bsv = -f32(f"{tag}_bf"), f32(f"{tag}_bs")  # [L, D]
            shared[f"cb_{rname}"] = np.ascontiguousarray(
                np.concatenate([bfv, bsv], axis=1).reshape(L, 1, 2 * D)).astype(BF)
    shared["nw_w"] = np.ascontiguousarray(f32("nw_W")).astype(BF)
    shared["nw_b"] = np.ascontiguousarray(f32("nw_b").reshape(L, 128, 1))
    shared["ident_f"] = np.eye(128, dtype=np.float32)
    shared["ident_b"] = np.eye(128).astype(BF)

    in_maps = []
    for c in range(CORES):
        m = dict(shared)
        m["xres_my"] = _dst_major_slice(x_my, c)
        m["xres_opp"] = _dst_major_slice(x_opp, c)
        m["xfm_my"] = _feat_major_slice(x_my, c)
        m["xfm_opp"] = _feat_major_slice(x_opp, c)
        for rname in packed:
            s_a, l_a = packed[rname][c]
            m[f"si_{rname}"] = _idx_dev(s_a)
            ohe, ohd = _onehots(l_a, Bmax)
            m[f"ohe_{rname}"] = ohe
            m[f"ohd_{rname}"] = ohd
        in_maps.append(m)

    trace = os.environ.get("KERNEL_PROFILE", "0") == "1"
    res = run_bass_kernel_spmd(nc, in_maps, core_ids=list(range(CORES)),
                               trace=trace, trace_cores=[0] if trace else None)
    LAST_EXEC_NS = res.exec_time_ns

    def unshard(key):
        # per-core [128 f, TILES*128 node] f32 -> [N, D]
        parts = []
        for c in range(CORES):
            a = res.results[c][key]  # [128, 2560]
            parts.append(np.ascontiguousarray(
                a.reshape(D, TILES, 128).transpose(1, 2, 0).reshape(SHARD, D)))
        return np.concatenate(parts)[:N]

    return unshard("out_my"), unshard("out_opp")


# revision 6
# speedup vs baseline: 3.1565x; 1.4147x over previous
"""Trainium2 Bass kernel for the 2-layer heterogeneous GNN (GATv2 + CGConv).

Sharding: destination nodes (both node types) are split into 8 contiguous
ranges of 2560 (N padded 20000 -> 20480); each core owns the edges that
target its range, for all 4 relations.  Node features are replicated
(SBUF-resident, bf16, node-wrapped layout) so per-edge source gathers are
SBUF->SBUF dma_gather ops (feature-major output); destination-side
per-edge values come from one-hot selector matmuls on the PE.  The
one-hot matrices (static, from the edge lists) are precomputed on the
host and streamed from HBM.  The inter-layer halo exchange is a single
AllGather of the updated 2560-row slices.
"""

import os
import numpy as np
import ml_dtypes

BF = ml_dtypes.bfloat16

N = 20000
D = 128
H = 4
L = 2
E = 80000
CORES = 8
NPAD = 20480
SHARD = 2560
TILES = 20           # dst tiles of 128 per core
RANKS = NPAD // 128  # 160
PAD_NODE = 20000     # zero-feature padding node (valid gather target)
GTILES = 4           # dst tiles per gather chunk

LAST_EXEC_NS = None

# relation table: (name, kind, src_type, dst_type); cg before gat per dst type
RELS = [
    ("loses", "cg", "my", "opp"),
    ("beats", "gat", "my", "opp"),
    ("rev_beats", "cg", "opp", "my"),
    ("rev_loses", "gat", "opp", "my"),
]


# ----------------------------------------------------------------- host prep

def _wrap_nodes(x):
    """[N,128] f32 -> node-wrapped [128, RANKS*128] bf16 (node n at
    partition n%128, cols (n//128)*128 : +128)."""
    xp = np.zeros((NPAD, D), np.float32)
    xp[:N] = x
    return np.ascontiguousarray(
        xp.reshape(RANKS, 128, D).transpose(1, 0, 2).reshape(128, RANKS * D)
    ).astype(BF)


def _dst_major_slice(x, c):
    """core c's own dst slice, dst-major [128, TILES*128] bf16."""
    xp = np.zeros((NPAD, D), np.float32)
    xp[:N] = x
    sl = xp[c * SHARD:(c + 1) * SHARD]
    return np.ascontiguousarray(
        sl.reshape(TILES, 128, D).transpose(1, 0, 2).reshape(128, TILES * D)
    ).astype(BF)


def _feat_major_slice(x, c):
    """core c's own dst slice, feature-major [128, TILES*128] bf16
    (col t*128+j = node c*2560+t*128+j)."""
    xp = np.zeros((NPAD, D), np.float32)
    xp[:N] = x
    sl = xp[c * SHARD:(c + 1) * SHARD]  # [2560, D]
    return np.ascontiguousarray(
        sl.reshape(TILES, 128, D).transpose(2, 0, 1).reshape(D, TILES * 128)
    ).astype(BF)


def _prep_edges(ei):
    """bucket edges by (core, dst tile); returns per-core lists + max count."""
    src = np.asarray(ei[0]).astype(np.int64)
    dst = np.asarray(ei[1]).astype(np.int64)
    percore = []
    maxcnt = 1
    for c in range(CORES):
        m = (dst >= c * SHARD) & (dst < (c + 1) * SHARD)
        s, d = src[m], dst[m]
        dl = d - c * SHARD
        tid = dl // 128
        buckets = [np.nonzero(tid == t)[0] for t in range(TILES)]
        for b in buckets:
            maxcnt = max(maxcnt, len(b))
        percore.append((s, dl, buckets))
    return percore, maxcnt


def _pack_edges(percore, Bmax):
    """-> per-core (src_ids [EP], dloc [EP]); EP = TILES*Bmax*128, pad=-1."""
    out = []
    for (s, dl, buckets) in percore:
        src_a = np.full((TILES, Bmax * 128), PAD_NODE, np.int64)
        loc_a = np.full((TILES, Bmax * 128), -1, np.int64)
        for t, b in enumerate(buckets):
            n = len(b)
            src_a[t, :n] = s[b]
            loc_a[t, :n] = dl[b] % 128
        out.append((src_a.reshape(-1), loc_a.reshape(-1)))
    return out


def _onehots(loc, Bmax):
    """loc [EP] (-1 = pad) -> (oh_e [128, NB*128], oh_d [128, NB*128]) bf16.

    oh_e block gb: [j=edge-in-block, d=dst-local]; oh_d block = transpose."""
    NB = TILES * Bmax
    EP = NB * 128
    M = np.zeros((EP, 128), np.float32)
    valid = loc >= 0
    M[np.nonzero(valid)[0], loc[valid]] = 1.0
    Mb = M.reshape(NB, 128, 128)
    oh_e = np.ascontiguousarray(Mb.transpose(1, 0, 2).reshape(128, NB * 128))
    oh_d = np.ascontiguousarray(Mb.transpose(2, 0, 1).reshape(128, NB * 128))
    return oh_e.astype(BF), oh_d.astype(BF)


def _idx_dev(a):
    """[EP] int -> [128, EP//16] int16 (16-partition wrap, replicated 8x)."""
    x = a.astype(np.int16).reshape(-1, 16).T
    return np.ascontiguousarray(np.tile(x, (8, 1)))


def _rep(v, rows=128):
    return np.ascontiguousarray(
        np.tile(np.asarray(v, np.float32).reshape(1, -1), (rows, 1)))


# ------------------------------------------------------------- program build

def _build_program(Bmax):
    import concourse.bass as bass
    import concourse.bacc as bacc
    import concourse.mybir as mybir
    import concourse.tile as tile
    from concourse.hw_specs import get_activation_tables

    F32, BF16, I16 = mybir.dt.float32, mybir.dt.bfloat16, mybir.dt.int16
    AF = mybir.ActivationFunctionType
    OP = mybir.AluOpType

    NB = TILES * Bmax
    EP = NB * 128
    EPQ = GTILES * Bmax * 128       # idxs per gather chunk
    NCH = TILES // GTILES           # gather chunks per relation
    CB = GTILES * Bmax              # blocks per chunk

    k_layers = int(os.environ.get("K_LAYERS", str(L)))
    k_rels = os.environ.get("K_RELS", "")
    rels_active = [r for r in RELS if (not k_rels or r[0] in k_rels.split(","))]

    nc = bacc.Bacc("TRN2", target_bir_lowering=False, debug=False,
                   num_devices=CORES)

    dr = {}
    dr["xw_my"] = nc.dram_tensor("xw_my", [128, RANKS * D], BF16, kind="ExternalInput")
    dr["xw_opp"] = nc.dram_tensor("xw_opp", [128, RANKS * D], BF16, kind="ExternalInput")
    for ty in ("my", "opp"):
        dr[f"xres_{ty}"] = nc.dram_tensor(f"xres_{ty}", [128, TILES * D], BF16, kind="ExternalInput")
        dr[f"xfm_{ty}"] = nc.dram_tensor(f"xfm_{ty}", [128, TILES * 128], BF16, kind="ExternalInput")
    for rname, kind, _, _ in RELS:
        dr[f"si_{rname}"] = nc.dram_tensor(f"si_{rname}", [128, EP // 16], I16, kind="ExternalInput")
        dr[f"ohe_{rname}"] = nc.dram_tensor(f"ohe_{rname}", [128, NB * 128], BF16, kind="ExternalInput")
        dr[f"ohd_{rname}"] = nc.dram_tensor(f"ohd_{rname}", [128, NB * 128], BF16, kind="ExternalInput")
        if kind == "gat":
            dr[f"wl_{rname}"] = nc.dram_tensor(f"wl_{rname}", [L, 128, H * D], BF16, kind="ExternalInput")
            dr[f"wr_{rname}"] = nc.dram_tensor(f"wr_{rname}", [L, 128, H * D], BF16, kind="ExternalInput")
            dr[f"att_{rname}"] = nc.dram_tensor(f"att_{rname}", [L, 128, H * D], BF16, kind="ExternalInput")
            dr[f"gb_{rname}"] = nc.dram_tensor(f"gb_{rname}", [L, 128, D], F32, kind="ExternalInput")
        else:
            dr[f"wt_{rname}"] = nc.dram_tensor(f"wt_{rname}", [L, 128, 2 * D], BF16, kind="ExternalInput")
            dr[f"wb_{rname}"] = nc.dram_tensor(f"wb_{rname}", [L, 128, 2 * D], BF16, kind="ExternalInput")
            dr[f"cb_{rname}"] = nc.dram_tensor(f"cb_{rname}", [L, 1, 2 * D], BF16, kind="ExternalInput")
    dr["nw_w"] = nc.dram_tensor("nw_w", [L, 128, D], BF16, kind="ExternalInput")
    dr["nw_b"] = nc.dram_tensor("nw_b", [L, 128, 1], F32, kind="ExternalInput")
    dr["ident_f"] = nc.dram_tensor("ident_f", [128, 128], F32, kind="ExternalInput")
    dr["ident_b"] = nc.dram_tensor("ident_b", [128, 128], BF16, kind="ExternalInput")
    dr["out_my"] = nc.dram_tensor("out_my", [128, TILES * D], F32, kind="ExternalOutput")
    dr["out_opp"] = nc.dram_tensor("out_opp", [128, TILES * D], F32, kind="ExternalOutput")

    def ld3(pool, name, src, cols, dt=None):
        t = pool.tile([128, L * cols], dt or src.dtype, name=name, tag=name)
        nc.sync.dma_start(
            t[:].rearrange("p (l n) -> p l n", l=L),
            src[:].rearrange("l p n -> p l n"),
        )
        return t

    with tile.TileContext(nc) as tc:
        with tc.tile_pool(name="const", bufs=1) as cst, \
             tc.tile_pool(name="xwp", bufs=1) as xwp, \
             tc.tile_pool(name="accp", bufs=1) as accp, \
             tc.tile_pool(name="gth", bufs=3) as gth, \
             tc.tile_pool(name="ohp", bufs=2) as ohp, \
             tc.tile_pool(name="wrk", bufs=3) as wrk, \
             tc.tile_pool(name="til", bufs=2) as til, \
             tc.tile_pool(name="epi", bufs=1) as epi, \
             tc.tile_pool(name="dram", bufs=1, space="DRAM") as drm, \
             tc.tile_pool(name="pz", bufs=4, space=bass.MemorySpace.PSUM) as pzp, \
             tc.tile_pool(name="pagg", bufs=2, space=bass.MemorySpace.PSUM) as paggp, \
             tc.tile_pool(name="pden", bufs=2, space=bass.MemorySpace.PSUM) as pdenp:

            # one activation table serves Exp/Ln/Prelu/Copy/Identity
            tabs = list(get_activation_tables(nc.m.arch).items())
            need = {AF.Exp, AF.Ln, AF.Prelu, AF.Copy, AF.Identity}
            set_id = next(i for i, (_, fns) in enumerate(tabs) if need <= fns)
            nc.scalar.add_instruction(mybir.InstLoadActFuncSet(
                name=nc.get_next_instruction_name(), ins=[], outs=[],
                act_func_set_id=set_id))

            # ---------------- persistent SBUF state
            xw = {}
            for ty in ("my", "opp"):
                xw[ty] = xwp.tile([128, RANKS * D], BF16, name=f"xw_{ty}_sb", tag=f"xw_{ty}_sb")
                nc.sync.dma_start(xw[ty][:], dr[f"xw_{ty}"][:])
            xres, xfm = {}, {}
            for ty in ("my", "opp"):
                xres[ty] = xwp.tile([128, TILES * D], BF16, name=f"xres_{ty}_sb", tag=f"xres_{ty}_sb")
                nc.sync.dma_start(xres[ty][:], dr[f"xres_{ty}"][:])
                xfm[ty] = xwp.tile([128, TILES * 128], BF16, name=f"xfm_{ty}_sb", tag=f"xfm_{ty}_sb")
                nc.sync.dma_start(xfm[ty][:], dr[f"xfm_{ty}"][:])

            cw = {}
            for rname, kind, _, _ in RELS:
                si = cst.tile([128, EP // 16], I16, name=f"si_{rname}_sb", tag=f"si_{rname}_sb")
                nc.sync.dma_start(si[:], dr[f"si_{rname}"][:])
                cw[rname] = {"si": si}
                if kind == "gat":
                    cw[rname]["wl"] = ld3(cst, f"wl_{rname}_sb", dr[f"wl_{rname}"], H * D)
                    cw[rname]["wr"] = ld3(cst, f"wr_{rname}_sb", dr[f"wr_{rname}"], H * D)
                    cw[rname]["att"] = ld3(cst, f"att_{rname}_sb", dr[f"att_{rname}"], H * D)
                    cw[rname]["gb"] = ld3(cst, f"gb_{rname}_sb", dr[f"gb_{rname}"], D)
                else:
                    cw[rname]["wt"] = ld3(cst, f"wt_{rname}_sb", dr[f"wt_{rname}"], 2 * D)
                    cw[rname]["wb"] = ld3(cst, f"wb_{rname}_sb", dr[f"wb_{rname}"], 2 * D)
                    cbt = cst.tile([1, L * 2 * D], BF16, name=f"cb_{rname}_sb", tag=f"cb_{rname}_sb")
                    nc.sync.dma_start(
                        cbt[:].rearrange("p (l n) -> p l n", l=L),
                        dr[f"cb_{rname}"][:].rearrange("l p n -> p l n"),
                    )
                    cw[rname]["cb"] = cbt
            nw_w = ld3(cst, "nw_w_sb", dr["nw_w"], D)
            nw_b = ld3(cst, "nw_b_sb", dr["nw_b"], 1)
            ident_f = cst.tile([128, 128], F32, name="identf_sb", tag="identf_sb")
            nc.sync.dma_start(ident_f[:], dr["ident_f"][:])
            ident_b = cst.tile([128, 128], BF16, name="identb_sb", tag="identb_sb")
            nc.sync.dma_start(ident_b[:], dr["ident_b"][:])
            ones_b = cst.tile([1, 128], BF16, name="ones_sb", tag="ones_sb")
            nc.gpsimd.memset(ones_b[:], 1.0)

            # ---------------- layers
            for l in range(k_layers):
                ACC = {}
                for ty in ("my", "opp"):
                    ACC[ty] = accp.tile([128, TILES * D], BF16, name=f"acc_{ty}_{l}", tag=f"acc_{ty}")

                for rname, kind, sty, dty in rels_active:
                    cwr = cw[rname]

                    # gather chunks (feature-major [128 f, EPQ e] bf16)
                    xsq = []
                    ohq = []
                    for q in range(NCH):
                        xs = gth.tile([128, EPQ], BF16, name=f"xs_{rname}_{l}_{q}", tag="xs")
                        nc.gpsimd.dma_gather(
                            out_ap=xs[:].rearrange("p (o n) -> p o n", o=1),
                            in_ap=xw[sty][:],
                            idxs_ap=cwr["si"][:, q * (EPQ // 16):(q + 1) * (EPQ // 16)],
                            num_idxs=EPQ, num_idxs_reg=EPQ,
                            elem_size=128, transpose=True,
                            single_packet=False,
                            sbuf_tokens_per_rank=128,
                            sbuf_free_dim_per_rank=256,
                            sbuf_free_dim_pad_per_rank=0,
                            sbuf_byte_offset=0,
                        )
                        xsq.append(xs)
                        ohe = ohp.tile([128, CB * 128], BF16, name=f"ohe_{rname}_{l}_{q}", tag="ohe")
                        nc.sync.dma_start(ohe[:], dr[f"ohe_{rname}"][:, q * CB * 128:(q + 1) * CB * 128])
                        ohd = ohp.tile([128, CB * 128], BF16, name=f"ohd_{rname}_{l}_{q}", tag="ohd")
                        nc.scalar.dma_start(ohd[:], dr[f"ohd_{rname}"][:, q * CB * 128:(q + 1) * CB * 128])
                        ohq.append((ohe, ohd))

                    for t in range(TILES):
                        q, tq = t // GTILES, t % GTILES
                        xs = xsq[q]
                        ohe_c, ohd_c = ohq[q]

                        # ---- per-tile dst-side precompute
                        if kind == "gat":
                            pxr = pzp.tile([128, H * D], F32, name=f"pxr_{rname}_{l}_{t}", tag="pz")
                            nc.tensor.matmul(pxr[:], xfm[dty][:, t * 128:(t + 1) * 128],
                                             cwr["wr"][:, l * H * D:(l + 1) * H * D],
                                             start=True, stop=True)
                            xr_sb = til.tile([128, H * D], BF16, name=f"xrsb_{rname}_{l}_{t}", tag="xr_sb")
                            nc.scalar.copy(xr_sb[:], pxr[:])
                            pagg = paggp.tile([128, H * D], F32, name=f"pagg_{rname}_{l}_{t}", tag="pagg")
                            pden = pdenp.tile([128, H], F32, name=f"pden_{rname}_{l}_{t}", tag="pden")
                        else:
                            pud = pzp.tile([128, 2 * D], F32, name=f"pud_{rname}_{l}_{t}", tag="pz")
                            nc.tensor.matmul(pud[:], xfm[dty][:, t * 128:(t + 1) * 128],
                                             cwr["wt"][:, l * 2 * D:(l + 1) * 2 * D],
                                             start=True, stop=False)
                            nc.tensor.matmul(pud[:], ones_b[:],
                                             cwr["cb"][:, l * 2 * D:(l + 1) * 2 * D],
                                             start=False, stop=True)
                            ud_sb = til.tile([128, 2 * D], BF16, name=f"udsb_{rname}_{l}_{t}", tag="ud_sb")
                            nc.scalar.copy(ud_sb[:], pud[:])
                            pagg = paggp.tile([128, D], F32, name=f"pagg_{rname}_{l}_{t}", tag="pagg")

                        for b in range(Bmax):
                            off = (tq * Bmax + b) * 128
                            xs_fm = xs[:, off:off + 128]
                            ohe = ohe_c[:, off:off + 128]
                            ohd = ohd_c[:, off:off + 128]
                            first, last = (b == 0), (b == Bmax - 1)

                            if kind == "gat":
                                # psz_l: Wl part only (for alpha-weighted aggregation)
                                psz_l = pzp.tile([128, H * D], F32, name=f"pszl_{l}_{t}_{b}", tag="pz")
                                nc.tensor.matmul(psz_l[:], xs_fm,
                                                 cwr["wl"][:, l * H * D:(l + 1) * H * D],
                                                 start=True, stop=True)
                                # psz_f: Wl + Wr[dst] (for the score)
                                psz_f = pzp.tile([128, H * D], F32, name=f"pszf_{l}_{t}_{b}", tag="pz")
                                nc.tensor.matmul(psz_f[:], ohd, xr_sb[:],
                                                 start=True, stop=False)
                                nc.tensor.matmul(psz_f[:], xs_fm,
                                                 cwr["wl"][:, l * H * D:(l + 1) * H * D],
                                                 start=False, stop=True)
                                z = wrk.tile([128, H * D], BF16, name=f"z_{l}_{t}_{b}", tag="z")
                                nc.scalar.activation(z[:], psz_f[:], AF.Prelu, alpha=0.2)
                                scp = wrk.tile([128, H * D], BF16, name=f"scp_{l}_{t}_{b}", tag="scp")
                                nc.vector.tensor_tensor(
                                    scp[:], z[:],
                                    cwr["att"][:, l * H * D:(l + 1) * H * D], op=OP.mult)
                                sc = wrk.tile([128, H], BF16, name=f"sc_{l}_{t}_{b}", tag="sc")
                                with nc.allow_low_precision(reason="softmax logits tolerate bf16"):
                                    nc.vector.tensor_reduce(
                                        sc[:], scp[:].rearrange("p (h f) -> p h f", f=D),
                                        axis=mybir.AxisListType.X, op=OP.add)
                                es = wrk.tile([128, H], BF16, name=f"es_{l}_{t}_{b}", tag="es")
                                nc.scalar.activation(es[:], sc[:], AF.Exp)
                                # xlw = xl * es[h]  (broadcast along feature dim)
                                xlw = wrk.tile([128, H * D], BF16, name=f"xlw_{l}_{t}_{b}", tag="xlw")
                                nc.vector.tensor_tensor(
                                    xlw[:].rearrange("p (h f) -> p h f", f=D),
                                    psz_l[:].rearrange("p (h f) -> p h f", f=D),
                                    es[:].unsqueeze(2).broadcast_to((128, H, D)),
                                    op=OP.mult)
                                nc.tensor.matmul(pagg[:], ohe, xlw[:], start=first, stop=last)
                                nc.tensor.matmul(pden[:], ohe, es[:], start=first, stop=last)
                            else:
                                psm = pzp.tile([128, 2 * D], F32, name=f"psm_{l}_{t}_{b}", tag="pz")
                                nc.tensor.matmul(psm[:], ohd, ud_sb[:],
                                                 start=True, stop=False)
                                nc.tensor.matmul(psm[:], xs_fm,
                                                 cwr["wb"][:, l * 2 * D:(l + 1) * 2 * D],
                                                 start=False, stop=True)
                                # cols 0:D hold -u; cols D:2D hold v (Wf negated on host)
                                s1 = wrk.tile([128, 2 * D], F32, name=f"s1_{l}_{t}_{b}", tag="s1")
                                nc.scalar.activation(s1[:], psm[:], AF.Exp)
                                sp = wrk.tile([128, D], F32, name=f"sp_{l}_{t}_{b}", tag="sp")
                                nc.scalar.activation(sp[:], s1[:, D:2 * D], AF.Ln, bias=1.0)
                                d1 = wrk.tile([128, D], F32, name=f"d1_{l}_{t}_{b}", tag="d1")
                                nc.scalar.activation(d1[:], s1[:, 0:D], AF.Identity, bias=1.0)
                                rsg = wrk.tile([128, D], F32, name=f"rsg_{l}_{t}_{b}", tag="rsg")
                                nc.vector.reciprocal_approx_fast(rsg[:], d1[:])
                                m = wrk.tile([128, D], BF16, name=f"m_{l}_{t}_{b}", tag="m")
                                nc.vector.tensor_tensor(m[:], rsg[:], sp[:], op=OP.mult)
                                nc.tensor.matmul(pagg[:], ohe, m[:], start=first, stop=last)

                        # -------- tile epilogue
                        asl = ACC[dty][:, t * D:(t + 1) * D]
                        if kind == "cg":
                            # ACC = cg_agg + residual  (cg runs first per type)
                            nc.vector.scalar_tensor_tensor(
                                asl, pagg[:], 1.0, xres[dty][:, t * D:(t + 1) * D],
                                op0=OP.mult, op1=OP.add)
                        else:
                            sden = til.tile([128, H], F32, name=f"sden_{rname}_{l}_{t}", tag="sden")
                            nc.vector.tensor_scalar(sden[:], pden[:], 1e-16, 4.0,
                                                    op0=OP.add, op1=OP.mult)
                            inv4 = til.tile([128, H], F32, name=f"inv4_{rname}_{l}_{t}", tag="inv4")
                            nc.vector.reciprocal_approx_fast(inv4[:], sden[:])
                            # gtmp = pagg * inv4[h] (broadcast), then sum heads
                            gtmp = til.tile([128, H * D], F32, name=f"gtmp_{rname}_{l}_{t}", tag="gtmp")
                            nc.vector.tensor_tensor(
                                gtmp[:].rearrange("p (h f) -> p h f", f=D),
                                pagg[:].rearrange("p (h f) -> p h f", f=D),
                                inv4[:].unsqueeze(2).broadcast_to((128, H, D)),
                                op=OP.mult)
                            gt = til.tile([128, D], F32, name=f"gt_{rname}_{l}_{t}", tag="gt")
                            nc.vector.tensor_reduce(
                                gt[:], gtmp[:].rearrange("p (h f) -> p f h", f=D),
                                axis=mybir.AxisListType.X, op=OP.add)
                            # ACC += gt + gb
                            gt2 = til.tile([128, D], F32, name=f"gt2_{rname}_{l}_{t}", tag="gt2")
                            nc.vector.scalar_tensor_tensor(
                                gt2[:], gt[:], 1.0, cwr["gb"][:, l * D:(l + 1) * D],
                                op0=OP.mult, op1=OP.add)
                            nc.vector.tensor_tensor(asl, asl, gt2[:], op=OP.add)

                # ---------------- layer epilogue: nodewise linear + layout
                last_layer = (l == k_layers - 1)
                if not last_layer:
                    ag_in = drm.tile([128, 2 * TILES * D], BF16, name=f"agin_{l}", tag="agin")
                    ag_out = drm.tile([CORES * 128, 2 * TILES * D], BF16,
                                      name=f"agout_{l}", tag="agout", addr_space="Shared")
                for tyi, ty in enumerate(("my", "opp")):
                    if ty not in {r[3] for r in rels_active}:
                        continue
                    accT = epi.tile([128, TILES * D], BF16, name=f"accT_{ty}_{l}", tag="accT")
                    for t in range(TILES):
                        ptr = pzp.tile([128, 128], BF16, name=f"ptr_{ty}_{l}_{t}", tag="pz")
                        nc.tensor.transpose(ptr[:], ACC[ty][:, t * D:(t + 1) * D], ident_b[:])
                        nc.scalar.copy(accT[:, t * D:(t + 1) * D], ptr[:])
                    for k in range(TILES * D // 512):
                        pnw = paggp.tile([128, 512], F32, name=f"pnw_{ty}_{l}_{k}", tag="pagg")
                        nc.tensor.matmul(pnw[:], nw_w[:, l * D:(l + 1) * D],
                                         accT[:, k * 512:(k + 1) * 512],
                                         start=True, stop=True)
                        if last_layer:
                            osb = epi.tile([128, 512], F32, name=f"osb_{ty}_{l}_{k}", tag="osb")
                            nc.scalar.activation(osb[:], pnw[:], AF.Identity,
                                                 bias=nw_b[:, l:l + 1])
                            nc.sync.dma_start(dr[f"out_{ty}"][:, k * 512:(k + 1) * 512], osb[:])
                        else:
                            nc.scalar.activation(xfm[ty][:, k * 512:(k + 1) * 512], pnw[:],
                                                 AF.Identity, bias=nw_b[:, l:l + 1])
                    if not last_layer:
                        # back to dst-major for residuals + halo exchange
                        for t in range(TILES):
                            ptr2 = pzp.tile([128, 128], BF16, name=f"ptr2_{ty}_{l}_{t}", tag="pz")
                            nc.tensor.transpose(ptr2[:], xfm[ty][:, t * D:(t + 1) * D], ident_b[:])
                            nc.vector.tensor_copy(xres[ty][:, t * D:(t + 1) * D], ptr2[:])
                        nc.sync.dma_start(
                            ag_in[:, tyi * TILES * D:(tyi + 1) * TILES * D], xres[ty][:])
                if not last_layer:
                    nc.gpsimd.collective_compute(
                        "AllGather", mybir.AluOpType.bypass,
                        replica_groups=[list(range(CORES))],
                        ins=[ag_in.opt()], outs=[ag_out.opt()],
                    )
                    for tyi, ty in enumerate(("my", "opp")):
                        nc.sync.dma_start(
                            xw[ty][:].rearrange("p (c j) -> p c j", c=CORES),
                            ag_out[:, tyi * TILES * D:(tyi + 1) * TILES * D]
                            .rearrange("(c p) j -> p c j", p=128),
                        )

    nc.compile()
    return nc


_prog_cache = {}


def _get_program(Bmax):
    if Bmax not in _prog_cache:
        _prog_cache[Bmax] = _build_program(Bmax)
    return _prog_cache[Bmax]


# ------------------------------------------------------------------- kernel

def kernel(**inputs):
    global LAST_EXEC_NS
    from concourse.bass_utils import run_bass_kernel_spmd

    f32 = lambda k: np.asarray(inputs[k], np.float32)
    x_my, x_opp = f32("x_my"), f32("x_opp")

    # edges
    eprep = {}
    Bmax = 1
    for rname, key in (("loses", "ei_loses"), ("beats", "ei_beats"),
                       ("rev_beats", "ei_rev_beats"), ("rev_loses", "ei_rev_loses")):
        percore, mc = _prep_edges(np.asarray(inputs[key]))
        eprep[rname] = percore
        Bmax = max(Bmax, -(-mc // 128))
    packed = {r: _pack_edges(eprep[r], Bmax) for r in eprep}

    nc = _get_program(Bmax)

    # shared (per-core identical) tensors
    shared = {}
    shared["xw_my"] = _wrap_nodes(x_my)
    shared["xw_opp"] = _wrap_nodes(x_opp)
    for rname, kind, _, _ in RELS:
        tag = {"loses": "cg_lose", "beats": "gat_beats",
               "rev_beats": "cg_rev", "rev_loses": "gat_rev"}[rname]
        if kind == "gat":
            shared[f"wl_{rname}"] = np.ascontiguousarray(f32(f"{tag}_Wl")).astype(BF)
            shared[f"wr_{rname}"] = np.ascontiguousarray(f32(f"{tag}_Wr")).astype(BF)
            att = f32(f"{tag}_att")  # [L, H, D]
            shared[f"att_{rname}"] = np.stack(
                [_rep(att[l].reshape(-1)) for l in range(L)]).astype(BF)
            b = f32(f"{tag}_b")  # [L, D]
            shared[f"gb_{rname}"] = np.stack([_rep(b[l]) for l in range(L)])
        else:
            # f-gate (Wf) negated so psm[:, :D] = -u and sigmoid(u) = 1/(1+e^{psm0})
            wf, ws = -f32(f"{tag}_Wf"), f32(f"{tag}_Ws")  # [L, 2D, D]
            shared[f"wt_{rname}"] = np.ascontiguousarray(
                np.concatenate([wf[:, :D, :], ws[:, :D, :]], axis=2)).astype(BF)
            shared[f"wb_{rname}"] = np.ascontiguousarray(
                np.concatenate([wf[:, D:, :], ws[:, D:, :]], axis=2)).astype(BF)
            bfv, bsv = -f32(f"{tag}_bf"), f32(f"{tag}_bs")  # [L, D]
            shared[f"cb_{rname}"] = np.ascontiguousarray(
                np.concatenate([bfv, bsv], axis=1).reshape(L, 1, 2 * D)).astype(BF)
    shared["nw_w"] = np.ascontiguousarray(f32("nw_W")).astype(BF)
    shared["nw_b"] = np.ascontiguousarray(f32("nw_b").reshape(L, 128, 1))
    shared["ident_f"] = np.eye(128, dtype=np.float32)
    shared["ident_b"] = np.eye(128).astype(BF)

    in_maps = []
    for c in range(CORES):
        m = dict(shared)
        m["xres_my"] = _dst_major_slice(x_my, c)
        m["xres_opp"] = _dst_major_slice(x_opp, c)
        m["xfm_my"] = _feat_major_slice(x_my, c)
        m["xfm_opp"] = _feat_major_slice(x_opp, c)
        for rname in packed:
            s_a, l_a = packed[rname][c]
            m[f"si_{rname}"] = _idx_dev(s_a)
            ohe, ohd = _onehots(l_a, Bmax)
            m[f"ohe_{rname}"] = ohe
            m[f"ohd_{rname}"] = ohd
        in_maps.append(m)

    trace = os.environ.get("KERNEL_PROFILE", "0") == "1"
    res = run_bass_kernel_spmd(nc, in_maps, core_ids=list(range(CORES)),
                               trace=trace, trace_cores=[0] if trace else None)
    LAST_EXEC_NS = res.exec_time_ns

    def unshard(key):
        # per-core [128 f, TILES*128 node] f32 -> [N, D]
        parts = []
        for c in range(CORES):
            a = res.results[c][key]  # [128, 2560]
            parts.append(np.ascontiguousarray(
                a.reshape(D, TILES, 128).transpose(1, 2, 0).reshape(SHARD, D)))
        return np.concatenate(parts)[:N]

    return unshard("out_my"), unshard("out_opp")


# revision 13
# speedup vs baseline: 3.1882x; 1.0100x over previous
"""Trainium2 Bass kernel for the 2-layer heterogeneous GNN (GATv2 + CGConv).

Sharding: destination nodes (both node types) are split into 8 contiguous
ranges of 2560 (N padded 20000 -> 20480); each core owns the edges that
target its range, for all 4 relations.  Node features are replicated
(SBUF-resident, bf16, node-wrapped layout) so per-edge source gathers are
SBUF->SBUF dma_gather ops (feature-major output); destination-side
per-edge values come from one-hot selector matmuls on the PE.  The
one-hot matrices (static, from the edge lists) are precomputed on the
host and streamed from HBM.  The inter-layer halo exchange is a single
AllGather of the updated 2560-row slices.
"""

import os
import numpy as np
import ml_dtypes

BF = ml_dtypes.bfloat16

N = 20000
D = 128
H = 4
L = 2
E = 80000
CORES = 8
NPAD = 20480
SHARD = 2560
TILES = 20           # dst tiles of 128 per core
RANKS = NPAD // 128  # 160
PAD_NODE = 20000     # zero-feature padding node (valid gather target)
GTILES = 4           # dst tiles per gather chunk

LAST_EXEC_NS = None
DBG = None

# relation table: (name, kind, src_type, dst_type); cg before gat per dst type
RELS = [
    ("loses", "cg", "my", "opp"),
    ("beats", "gat", "my", "opp"),
    ("rev_beats", "cg", "opp", "my"),
    ("rev_loses", "gat", "opp", "my"),
]


# ----------------------------------------------------------------- host prep

def _wrap_nodes(x):
    """[N,128] f32 -> node-wrapped [128, RANKS*128] bf16 (node n at
    partition n%128, cols (n//128)*128 : +128)."""
    xp = np.zeros((NPAD, D), np.float32)
    xp[:N] = x
    return np.ascontiguousarray(
        xp.reshape(RANKS, 128, D).transpose(1, 0, 2).reshape(128, RANKS * D)
    ).astype(BF)


def _dst_major_slice(x, c):
    """core c's own dst slice, dst-major [128, TILES*128] bf16."""
    xp = np.zeros((NPAD, D), np.float32)
    xp[:N] = x
    sl = xp[c * SHARD:(c + 1) * SHARD]
    return np.ascontiguousarray(
        sl.reshape(TILES, 128, D).transpose(1, 0, 2).reshape(128, TILES * D)
    ).astype(BF)


def _feat_major_slice(x, c):
    """core c's own dst slice, feature-major [128, TILES*128] bf16
    (col t*128+j = node c*2560+t*128+j)."""
    xp = np.zeros((NPAD, D), np.float32)
    xp[:N] = x
    sl = xp[c * SHARD:(c + 1) * SHARD]  # [2560, D]
    return np.ascontiguousarray(
        sl.reshape(TILES, 128, D).transpose(2, 0, 1).reshape(D, TILES * 128)
    ).astype(BF)


def _prep_edges(ei):
    """bucket edges by (core, dst tile); returns per-core lists + max count."""
    src = np.asarray(ei[0]).astype(np.int64)
    dst = np.asarray(ei[1]).astype(np.int64)
    percore = []
    maxcnt = 1
    for c in range(CORES):
        m = (dst >= c * SHARD) & (dst < (c + 1) * SHARD)
        s, d = src[m], dst[m]
        dl = d - c * SHARD
        tid = dl // 128
        buckets = [np.nonzero(tid == t)[0] for t in range(TILES)]
        for b in buckets:
            maxcnt = max(maxcnt, len(b))
        percore.append((s, dl, buckets))
    return percore, maxcnt


def _pack_edges(percore, Bmax):
    """-> per-core (src_ids [EP], dloc [EP]); EP = TILES*Bmax*128, pad=-1."""
    out = []
    for (s, dl, buckets) in percore:
        src_a = np.full((TILES, Bmax * 128), PAD_NODE, np.int64)
        loc_a = np.full((TILES, Bmax * 128), -1, np.int64)
        for t, b in enumerate(buckets):
            n = len(b)
            src_a[t, :n] = s[b]
            loc_a[t, :n] = dl[b] % 128
        out.append((src_a.reshape(-1), loc_a.reshape(-1)))
    return out


def _onehots(loc, Bmax):
    """loc [EP] (-1 = pad) -> (oh_e [128, NB*128], oh_d [128, NB*128]) bf16.

    oh_e block gb: [j=edge-in-block, d=dst-local]; oh_d block = transpose."""
    NB = TILES * Bmax
    EP = NB * 128
    M = np.zeros((EP, 128), np.float32)
    valid = loc >= 0
    M[np.nonzero(valid)[0], loc[valid]] = 1.0
    Mb = M.reshape(NB, 128, 128)
    oh_e = np.ascontiguousarray(Mb.transpose(1, 0, 2).reshape(128, NB * 128))
    oh_d = np.ascontiguousarray(Mb.transpose(2, 0, 1).reshape(128, NB * 128))
    return oh_e.astype(BF), oh_d.astype(BF)


def _idx_dev(a):
    """[EP] int -> [128, EP//16] int16 (16-partition wrap, replicated 8x)."""
    x = a.astype(np.int16).reshape(-1, 16).T
    return np.ascontiguousarray(np.tile(x, (8, 1)))


def _rep(v, rows=128):
    return np.ascontiguousarray(
        np.tile(np.asarray(v, np.float32).reshape(1, -1), (rows, 1)))


# ------------------------------------------------------------- program build

def _build_program(Bmax):
    import concourse.bass as bass
    import concourse.bacc as bacc
    import concourse.mybir as mybir
    import concourse.tile as tile
    from concourse.hw_specs import get_activation_tables

    F32, BF16, I16 = mybir.dt.float32, mybir.dt.bfloat16, mybir.dt.int16
    AF = mybir.ActivationFunctionType
    OP = mybir.AluOpType

    NB = TILES * Bmax
    EP = NB * 128
    EPQ = GTILES * Bmax * 128       # idxs per gather chunk
    NCH = TILES // GTILES           # gather chunks per relation
    CB = GTILES * Bmax              # blocks per chunk

    k_layers = int(os.environ.get("K_LAYERS", str(L)))
    k_rels = os.environ.get("K_RELS", "")
    rels_active = [r for r in RELS if (not k_rels or r[0] in k_rels.split(","))]

    nc = bacc.Bacc("TRN2", target_bir_lowering=False, debug=False,
                   num_devices=CORES)

    dr = {}
    dr["xw_my"] = nc.dram_tensor("xw_my", [128, RANKS * D], BF16, kind="ExternalInput")
    dr["xw_opp"] = nc.dram_tensor("xw_opp", [128, RANKS * D], BF16, kind="ExternalInput")
    for ty in ("my", "opp"):
        dr[f"xres_{ty}"] = nc.dram_tensor(f"xres_{ty}", [128, TILES * D], BF16, kind="ExternalInput")
        dr[f"xfm_{ty}"] = nc.dram_tensor(f"xfm_{ty}", [128, TILES * 128], BF16, kind="ExternalInput")
    for rname, kind, _, _ in RELS:
        dr[f"si_{rname}"] = nc.dram_tensor(f"si_{rname}", [128, EP // 16], I16, kind="ExternalInput")
        dr[f"ohe_{rname}"] = nc.dram_tensor(f"ohe_{rname}", [128, NB * 128], BF16, kind="ExternalInput")
        dr[f"ohd_{rname}"] = nc.dram_tensor(f"ohd_{rname}", [128, NB * 128], BF16, kind="ExternalInput")
        if kind == "gat":
            dr[f"wl_{rname}"] = nc.dram_tensor(f"wl_{rname}", [L, 128, H * D], BF16, kind="ExternalInput")
            dr[f"wr_{rname}"] = nc.dram_tensor(f"wr_{rname}", [L, 128, H * D], BF16, kind="ExternalInput")
            dr[f"att_{rname}"] = nc.dram_tensor(f"att_{rname}", [L, 128, H * D], BF16, kind="ExternalInput")
            dr[f"gb_{rname}"] = nc.dram_tensor(f"gb_{rname}", [L, 128, D], F32, kind="ExternalInput")
        else:
            dr[f"wt_{rname}"] = nc.dram_tensor(f"wt_{rname}", [L, 128, 2 * D], BF16, kind="ExternalInput")
            dr[f"wb_{rname}"] = nc.dram_tensor(f"wb_{rname}", [L, 128, 2 * D], BF16, kind="ExternalInput")
            dr[f"cb_{rname}"] = nc.dram_tensor(f"cb_{rname}", [L, 1, 2 * D], BF16, kind="ExternalInput")
    dr["nw_w"] = nc.dram_tensor("nw_w", [L, 128, D], BF16, kind="ExternalInput")
    dr["nw_b"] = nc.dram_tensor("nw_b", [L, 128, 1], F32, kind="ExternalInput")
    dr["ident_f"] = nc.dram_tensor("ident_f", [128, 128], F32, kind="ExternalInput")
    dr["ident_b"] = nc.dram_tensor("ident_b", [128, 128], BF16, kind="ExternalInput")
    dr["out_my"] = nc.dram_tensor("out_my", [128, TILES * D], F32, kind="ExternalOutput")

    dr["out_opp"] = nc.dram_tensor("out_opp", [128, TILES * D], F32, kind="ExternalOutput")

    def ld3(pool, name, src, cols, dt=None):
        t = pool.tile([128, L * cols], dt or src.dtype, name=name, tag=name)
        nc.sync.dma_start(
            t[:].rearrange("p (l n) -> p l n", l=L),
            src[:].rearrange("l p n -> p l n"),
        )
        return t

    with tile.TileContext(nc) as tc:
        with tc.tile_pool(name="const", bufs=1) as cst, \
             tc.tile_pool(name="xwp", bufs=1) as xwp, \
             tc.tile_pool(name="accp", bufs=1) as accp, \
             tc.tile_pool(name="gth", bufs=3) as gth, \
             tc.tile_pool(name="ohp", bufs=2) as ohp, \
             tc.tile_pool(name="wrk", bufs=3) as wrk, \
             tc.tile_pool(name="til", bufs=2) as til, \
             tc.tile_pool(name="epi", bufs=1) as epi, \
             tc.tile_pool(name="dram", bufs=1, space="DRAM") as drm, \
             tc.tile_pool(name="pz", bufs=5, space=bass.MemorySpace.PSUM) as pzp, \
             tc.tile_pool(name="pagg", bufs=2, space=bass.MemorySpace.PSUM) as paggp:

            # one activation table serves Exp/Ln/Prelu/Copy/Identity
            tabs = list(get_activation_tables(nc.m.arch).items())
            need = {AF.Exp, AF.Ln, AF.Prelu, AF.Copy, AF.Identity}
            set_id = next(i for i, (_, fns) in enumerate(tabs) if need <= fns)
            nc.scalar.add_instruction(mybir.InstLoadActFuncSet(
                name=nc.get_next_instruction_name(), ins=[], outs=[],
                act_func_set_id=set_id))

            # ---------------- persistent SBUF state
            xw = {}
            for ty in ("my", "opp"):
                xw[ty] = xwp.tile([128, RANKS * D], BF16, name=f"xw_{ty}_sb", tag=f"xw_{ty}_sb")
                nc.sync.dma_start(xw[ty][:], dr[f"xw_{ty}"][:])
            xres, xfm = {}, {}
            for ty in ("my", "opp"):
                xres[ty] = xwp.tile([128, TILES * D], BF16, name=f"xres_{ty}_sb", tag=f"xres_{ty}_sb")
                nc.sync.dma_start(xres[ty][:], dr[f"xres_{ty}"][:])
                xfm[ty] = xwp.tile([128, TILES * 128], BF16, name=f"xfm_{ty}_sb", tag=f"xfm_{ty}_sb")
                nc.sync.dma_start(xfm[ty][:], dr[f"xfm_{ty}"][:])

            cw = {}
            for rname, kind, _, _ in RELS:
                si = cst.tile([128, EP // 16], I16, name=f"si_{rname}_sb", tag=f"si_{rname}_sb")
                nc.sync.dma_start(si[:], dr[f"si_{rname}"][:])
                cw[rname] = {"si": si}
                if kind == "gat":
                    cw[rname]["wl"] = ld3(cst, f"wl_{rname}_sb", dr[f"wl_{rname}"], H * D)
                    cw[rname]["wr"] = ld3(cst, f"wr_{rname}_sb", dr[f"wr_{rname}"], H * D)
                    cw[rname]["att"] = ld3(cst, f"att_{rname}_sb", dr[f"att_{rname}"], H * D)
                    cw[rname]["gb"] = ld3(cst, f"gb_{rname}_sb", dr[f"gb_{rname}"], D)
                else:
                    cw[rname]["wt"] = ld3(cst, f"wt_{rname}_sb", dr[f"wt_{rname}"], 2 * D)
                    cw[rname]["wb"] = ld3(cst, f"wb_{rname}_sb", dr[f"wb_{rname}"], 2 * D)
                    cbt = cst.tile([1, L * 2 * D], BF16, name=f"cb_{rname}_sb", tag=f"cb_{rname}_sb")
                    nc.sync.dma_start(
                        cbt[:].rearrange("p (l n) -> p l n", l=L),
                        dr[f"cb_{rname}"][:].rearrange("l p n -> p l n"),
                    )
                    cw[rname]["cb"] = cbt
            nw_w = ld3(cst, "nw_w_sb", dr["nw_w"], D)
            nw_b = ld3(cst, "nw_b_sb", dr["nw_b"], 1)
            ident_f = cst.tile([128, 128], F32, name="identf_sb", tag="identf_sb")
            nc.sync.dma_start(ident_f[:], dr["ident_f"][:])
            ident_b = cst.tile([128, 128], BF16, name="identb_sb", tag="identb_sb")
            nc.sync.dma_start(ident_b[:], dr["ident_b"][:])
            ones_b = cst.tile([1, 128], BF16, name="ones_sb", tag="ones_sb")
            nc.gpsimd.memset(ones_b[:], 1.0)

            # ---------------- layers
            def gather_chunk(rname, sty, l, q):
                cwr = cw[rname]
                xs = gth.tile([128, EPQ], BF16, name=f"xs_{rname}_{l}_{q}", tag="xs")
                nc.gpsimd.dma_gather(
                    out_ap=xs[:].rearrange("p (o n) -> p o n", o=1),
                    in_ap=xw[sty][:],
                    idxs_ap=cwr["si"][:, q * (EPQ // 16):(q + 1) * (EPQ // 16)],
                    num_idxs=EPQ, num_idxs_reg=EPQ,
                    elem_size=128, transpose=True,
                    single_packet=False,
                    sbuf_tokens_per_rank=128,
                    sbuf_free_dim_per_rank=256,
                    sbuf_free_dim_pad_per_rank=0,
                    sbuf_byte_offset=0,
                )
                ohe = ohp.tile([128, CB * 128], BF16, name=f"ohe_{rname}_{l}_{q}", tag="ohe")
                nc.sync.dma_start(ohe[:], dr[f"ohe_{rname}"][:, q * CB * 128:(q + 1) * CB * 128])
                ohd = ohp.tile([128, CB * 128], BF16, name=f"ohd_{rname}_{l}_{q}", tag="ohd")
                nc.scalar.dma_start(ohd[:], dr[f"ohd_{rname}"][:, q * CB * 128:(q + 1) * CB * 128])
                return xs, ohe, ohd

            def gat_tile(rname, dty, l, t, tq, ACC, xs, ohe_c, ohd_c):
                cwr = cw[rname]
                pxr = pzp.tile([128, H * D], F32, name=f"pxr_{rname}_{l}_{t}", tag="pz")
                nc.tensor.matmul(pxr[:], xfm[dty][:, t * 128:(t + 1) * 128],
                                 cwr["wr"][:, l * H * D:(l + 1) * H * D],
                                 start=True, stop=True)
                xr_sb = til.tile([128, H * D], BF16, name=f"xrsb_{rname}_{l}_{t}", tag="xr_sb")
                nc.scalar.copy(xr_sb[:], pxr[:])
                pagg = paggp.tile([128, H * D], F32, name=f"pagg_{rname}_{l}_{t}", tag="pagg")
                pden = paggp.tile([128, H], F32, name=f"pden_{rname}_{l}_{t}", tag="pden")

                pend = []
                for b in range(Bmax):
                    off = (tq * Bmax + b) * 128
                    xs_fm = xs[:, off:off + 128]
                    ohe = ohe_c[:, off:off + 128]
                    ohd = ohd_c[:, off:off + 128]
                    first, last = (b == 0), (b == Bmax - 1)
                    # psz_l: Wl part only (for alpha-weighted aggregation)
                    psz_l = pzp.tile([128, H * D], F32, name=f"pszl_{rname}_{l}_{t}_{b}", tag="pz")
                    nc.tensor.matmul(psz_l[:], xs_fm,
                                     cwr["wl"][:, l * H * D:(l + 1) * H * D],
                                     start=True, stop=True)
                    # psz_f: Wl + Wr[dst] (for the score)
                    psz_f = pzp.tile([128, H * D], F32, name=f"pszf_{rname}_{l}_{t}_{b}", tag="pz")
                    nc.tensor.matmul(psz_f[:], ohd, xr_sb[:],
                                     start=True, stop=False)
                    nc.tensor.matmul(psz_f[:], xs_fm,
                                     cwr["wl"][:, l * H * D:(l + 1) * H * D],
                                     start=False, stop=True)
                    z = wrk.tile([128, H * D], BF16, name=f"z_{rname}_{l}_{t}_{b}", tag="z")
                    nc.scalar.activation(z[:], psz_f[:], AF.Prelu, alpha=0.2)
                    scp = wrk.tile([128, H * D], BF16, name=f"scp_{rname}_{l}_{t}_{b}", tag="scp")
                    nc.vector.tensor_tensor(
                        scp[:], z[:],
                        cwr["att"][:, l * H * D:(l + 1) * H * D], op=OP.mult)
                    sc = wrk.tile([128, H], BF16, name=f"sc_{rname}_{l}_{t}_{b}", tag="sc")
                    with nc.allow_low_precision(reason="softmax logits tolerate bf16"):
                        nc.vector.tensor_reduce(
                            sc[:], scp[:].rearrange("p (h f) -> p h f", f=D),
                            axis=mybir.AxisListType.X, op=OP.add)
                    es = wrk.tile([128, H], BF16, name=f"es_{rname}_{l}_{t}_{b}", tag="es")
                    nc.scalar.activation(es[:], sc[:], AF.Exp)
                    # xlw = xl * es[h]  (broadcast along feature dim)
                    xlw = wrk.tile([128, H * D], BF16, name=f"xlw_{rname}_{l}_{t}_{b}", tag="xlw")
                    nc.vector.tensor_tensor(
                        xlw[:].rearrange("p (h f) -> p h f", f=D),
                        psz_l[:].rearrange("p (h f) -> p h f", f=D),
                        es[:].unsqueeze(2).broadcast_to((128, H, D)),
                        op=OP.mult)
                    if pend:
                        pend.pop(0)()
                    pend.append(
                        (lambda ohe=ohe, xlw=xlw, es=es, first=first, last=last: (
                            nc.tensor.matmul(pagg[:], ohe, xlw[:], start=first, stop=last),
                            nc.tensor.matmul(pden[:], ohe, es[:], start=first, stop=last))))
                while pend:
                    pend.pop(0)()

                # -------- tile epilogue (gat adds after cg wrote ACC)
                asl = ACC[dty][:, t * D:(t + 1) * D]
                sden = til.tile([128, H], F32, name=f"sden_{rname}_{l}_{t}", tag="sden")
                nc.vector.tensor_scalar(sden[:], pden[:], 1e-16, 4.0,
                                        op0=OP.add, op1=OP.mult)
                inv4 = til.tile([128, H], F32, name=f"inv4_{rname}_{l}_{t}", tag="inv4")
                nc.vector.reciprocal_approx_fast(inv4[:], sden[:])
                gtmp = til.tile([128, H * D], F32, name=f"gtmp_{rname}_{l}_{t}", tag="gtmp")
                nc.vector.tensor_tensor(
                    gtmp[:].rearrange("p (h f) -> p h f", f=D),
                    pagg[:].rearrange("p (h f) -> p h f", f=D),
                    inv4[:].unsqueeze(2).broadcast_to((128, H, D)),
                    op=OP.mult)
                gt = til.tile([128, D], F32, name=f"gt_{rname}_{l}_{t}", tag="gt")
                nc.vector.tensor_reduce(
                    gt[:], gtmp[:].rearrange("p (h f) -> p f h", f=D),
                    axis=mybir.AxisListType.X, op=OP.add)
                gt2 = til.tile([128, D], F32, name=f"gt2_{rname}_{l}_{t}", tag="gt2")
                nc.vector.scalar_tensor_tensor(
                    gt2[:], gt[:], 1.0, cwr["gb"][:, l * D:(l + 1) * D],
                    op0=OP.mult, op1=OP.add)
                nc.vector.tensor_tensor(asl, asl, gt2[:], op=OP.add)

            def cg_tile(rname, dty, l, t, tq, ACC, xs, ohe_c, ohd_c):
                cwr = cw[rname]
                pud = pzp.tile([128, 2 * D], F32, name=f"pud_{rname}_{l}_{t}", tag="pz")
                nc.tensor.matmul(pud[:], xfm[dty][:, t * 128:(t + 1) * 128],
                                 cwr["wt"][:, l * 2 * D:(l + 1) * 2 * D],
                                 start=True, stop=False)
                nc.tensor.matmul(pud[:], ones_b[:],
                                 cwr["cb"][:, l * 2 * D:(l + 1) * 2 * D],
                                 start=False, stop=True)
                ud_sb = til.tile([128, 2 * D], BF16, name=f"udsb_{rname}_{l}_{t}", tag="ud_sb")
                nc.scalar.copy(ud_sb[:], pud[:])
                pagg = paggp.tile([128, D], F32, name=f"pagg_{rname}_{l}_{t}", tag="pagg")

                pend = []
                for b in range(Bmax):
                    off = (tq * Bmax + b) * 128
                    xs_fm = xs[:, off:off + 128]
                    ohe = ohe_c[:, off:off + 128]
                    ohd = ohd_c[:, off:off + 128]
                    first, last = (b == 0), (b == Bmax - 1)
                    psm = pzp.tile([128, 2 * D], F32, name=f"psm_{rname}_{l}_{t}_{b}", tag="pz")
                    nc.tensor.matmul(psm[:], ohd, ud_sb[:],
                                     start=True, stop=False)
                    nc.tensor.matmul(psm[:], xs_fm,
                                     cwr["wb"][:, l * 2 * D:(l + 1) * 2 * D],
                                     start=False, stop=True)
                    # cols 0:D hold -u; cols D:2D hold v (Wf negated on host)
                    s1 = wrk.tile([128, 2 * D], F32, name=f"s1_{rname}_{l}_{t}_{b}", tag="s1")
                    nc.scalar.activation(s1[:], psm[:], AF.Exp)
                    sp = wrk.tile([128, D], F32, name=f"sp_{rname}_{l}_{t}_{b}", tag="sp")
                    nc.scalar.activation(sp[:], s1[:, D:2 * D], AF.Ln, bias=1.0)
                    d1 = wrk.tile([128, D], F32, name=f"d1_{rname}_{l}_{t}_{b}", tag="d1")
                    nc.scalar.activation(d1[:], s1[:, 0:D], AF.Identity, bias=1.0)
                    rsg = wrk.tile([128, D], F32, name=f"rsg_{rname}_{l}_{t}_{b}", tag="rsg")
                    nc.vector.reciprocal_approx_fast(rsg[:], d1[:])
                    m = wrk.tile([128, D], BF16, name=f"m_{rname}_{l}_{t}_{b}", tag="m")
                    nc.vector.tensor_tensor(m[:], rsg[:], sp[:], op=OP.mult)
                    if len(pend) >= 2:
                        pend.pop(0)()
                    pend.append(
                        (lambda ohe=ohe, m=m, first=first, last=last:
                            nc.tensor.matmul(pagg[:], ohe, m[:], start=first, stop=last)))
                while pend:
                    pend.pop(0)()

                # -------- tile epilogue: ACC = cg_agg + residual (cg first)
                asl = ACC[dty][:, t * D:(t + 1) * D]
                nc.vector.scalar_tensor_tensor(
                    asl, pagg[:], 1.0, xres[dty][:, t * D:(t + 1) * D],
                    op0=OP.mult, op1=OP.add)

            def type_epilogue(ty, tyi, l, ACC, last_layer):
                accT = epi.tile([128, TILES * D], BF16, name=f"accT_{ty}_{l}", tag="accT")
                for t in range(TILES):
                    ptr = pzp.tile([128, 128], BF16, name=f"ptr_{ty}_{l}_{t}", tag="pz")
                    nc.tensor.transpose(ptr[:], ACC[ty][:, t * D:(t + 1) * D], ident_b[:])
                    nc.scalar.copy(accT[:, t * D:(t + 1) * D], ptr[:])
                for k in range(TILES * D // 512):
                    pnw = paggp.tile([128, 512], F32, name=f"pnw_{ty}_{l}_{k}", tag="pagg")
                    nc.tensor.matmul(pnw[:], nw_w[:, l * D:(l + 1) * D],
                                     accT[:, k * 512:(k + 1) * 512],
                                     start=True, stop=True)
                    if last_layer:
                        osb = epi.tile([128, 512], F32, name=f"osb_{ty}_{l}_{k}", tag="osb")
                        nc.scalar.activation(osb[:], pnw[:], AF.Identity,
                                             bias=nw_b[:, l:l + 1])
                        nc.sync.dma_start(dr[f"out_{ty}"][:, k * 512:(k + 1) * 512], osb[:])
                    else:
                        nc.scalar.activation(xfm[ty][:, k * 512:(k + 1) * 512], pnw[:],
                                             AF.Identity, bias=nw_b[:, l:l + 1])
                if not last_layer:
                    # back to dst-major for residuals + halo exchange
                    for t in range(TILES):
                        ptr2 = pzp.tile([128, 128], BF16, name=f"ptr2_{ty}_{l}_{t}", tag="pz")
                        nc.tensor.transpose(ptr2[:], xfm[ty][:, t * D:(t + 1) * D], ident_b[:])
                        nc.vector.tensor_copy(xres[ty][:, t * D:(t + 1) * D], ptr2[:])
                    ag_in = drm.tile([128, TILES * D], BF16, name=f"agin_{ty}_{l}", tag=f"agin_{ty}")
                    ag_out = drm.tile([CORES * 128, TILES * D], BF16,
                                      name=f"agout_{ty}_{l}", tag=f"agout_{ty}",
                                      addr_space="Shared")
                    nc.sync.dma_start(ag_in[:], xres[ty][:])
                    agins[ty] = (ag_in, ag_out)

            agins = {}
            for l in range(k_layers):
                last_layer = (l == k_layers - 1)
                ACC = {}
                for ty in ("my", "opp"):
                    ACC[ty] = accp.tile([128, TILES * D], BF16, name=f"acc_{ty}_{l}", tag=f"acc_{ty}")

                for rname, kind, sty, dty in rels_active:
                    chunks = [gather_chunk(rname, sty, l, q) for q in range(NCH)]
                    for t in range(TILES):
                        q, tq = t // GTILES, t % GTILES
                        if kind == "cg":
                            cg_tile(rname, dty, l, t, tq, ACC, *chunks[q])
                        else:
                            gat_tile(rname, dty, l, t, tq, ACC, *chunks[q])

                for tyi, ty in enumerate(("my", "opp")):
                    if ty not in {r[3] for r in rels_active}:
                        continue
                    type_epilogue(ty, tyi, l, ACC, last_layer)
                if not last_layer:
                    for ty in ("my", "opp"):
                        ag_in, ag_out = agins.pop(ty)
                        nc.gpsimd.collective_compute(
                            "AllGather", mybir.AluOpType.bypass,
                            replica_groups=[list(range(CORES))],
                            ins=[ag_in.opt()], outs=[ag_out.opt()],
                        )
                        nc.sync.dma_start(
                            xw[ty][:].rearrange("p (c j) -> p c j", c=CORES),
                            ag_out[:].rearrange("(c p) j -> p c j", p=128),
                        )


    nc.compile()
    return nc


_prog_cache = {}


def _get_program(Bmax):
    if Bmax not in _prog_cache:
        _prog_cache[Bmax] = _build_program(Bmax)
    return _prog_cache[Bmax]


# ------------------------------------------------------------------- kernel

def kernel(**inputs):
    global LAST_EXEC_NS
    from concourse.bass_utils import run_bass_kernel_spmd

    f32 = lambda k: np.asarray(inputs[k], np.float32)
    x_my, x_opp = f32("x_my"), f32("x_opp")

    # edges
    eprep = {}
    Bmax = 1
    for rname, key in (("loses", "ei_loses"), ("beats", "ei_beats"),
                       ("rev_beats", "ei_rev_beats"), ("rev_loses", "ei_rev_loses")):
        percore, mc = _prep_edges(np.asarray(inputs[key]))
        eprep[rname] = percore
        Bmax = max(Bmax, -(-mc // 128))
    packed = {r: _pack_edges(eprep[r], Bmax) for r in eprep}

    nc = _get_program(Bmax)

    # shared (per-core identical) tensors
    shared = {}
    shared["xw_my"] = _wrap_nodes(x_my)
    shared["xw_opp"] = _wrap_nodes(x_opp)
    for rname, kind, _, _ in RELS:
        tag = {"loses": "cg_lose", "beats": "gat_beats",
               "rev_beats": "cg_rev", "rev_loses": "gat_rev"}[rname]
        if kind == "gat":
            shared[f"wl_{rname}"] = np.ascontiguousarray(f32(f"{tag}_Wl")).astype(BF)
            shared[f"wr_{rname}"] = np.ascontiguousarray(f32(f"{tag}_Wr")).astype(BF)
            att = f32(f"{tag}_att")  # [L, H, D]
            shared[f"att_{rname}"] = np.stack(
                [_rep(att[l].reshape(-1)) for l in range(L)]).astype(BF)
            b = f32(f"{tag}_b")  # [L, D]
            shared[f"gb_{rname}"] = np.stack([_rep(b[l]) for l in range(L)])
        else:
            # f-gate (Wf) negated so psm[:, :D] = -u and sigmoid(u) = 1/(1+e^{psm0})
            wf, ws = -f32(f"{tag}_Wf"), f32(f"{tag}_Ws")  # [L, 2D, D]
            shared[f"wt_{rname}"] = np.ascontiguousarray(
                np.concatenate([wf[:, :D, :], ws[:, :D, :]], axis=2)).astype(BF)
            shared[f"wb_{rname}"] = np.ascontiguousarray(
                np.concatenate([wf[:, D:, :], ws[:, D:, :]], axis=2)).astype(BF)
            bfv, bsv = -f32(f"{tag}_bf"), f32(f"{tag}_bs")  # [L, D]
            shared[f"cb_{rname}"] = np.ascontiguousarray(
                np.concatenate([bfv, bsv], axis=1).reshape(L, 1, 2 * D)).astype(BF)
    shared["nw_w"] = np.ascontiguousarray(f32("nw_W")).astype(BF)
    shared["nw_b"] = np.ascontiguousarray(f32("nw_b").reshape(L, 128, 1))
    shared["ident_f"] = np.eye(128, dtype=np.float32)
    shared["ident_b"] = np.eye(128).astype(BF)

    in_maps = []
    for c in range(CORES):
        m = dict(shared)
        m["xres_my"] = _dst_major_slice(x_my, c)
        m["xres_opp"] = _dst_major_slice(x_opp, c)
        m["xfm_my"] = _feat_major_slice(x_my, c)
        m["xfm_opp"] = _feat_major_slice(x_opp, c)
        for rname in packed:
            s_a, l_a = packed[rname][c]
            m[f"si_{rname}"] = _idx_dev(s_a)
            ohe, ohd = _onehots(l_a, Bmax)
            m[f"ohe_{rname}"] = ohe
            m[f"ohd_{rname}"] = ohd
        in_maps.append(m)

    trace = os.environ.get("KERNEL_PROFILE", "0") == "1"
    res = run_bass_kernel_spmd(nc, in_maps, core_ids=list(range(CORES)),
                               trace=trace, trace_cores=[0] if trace else None)
    LAST_EXEC_NS = res.exec_time_ns

    global DBG
    DBG = res.results

    def unshard(key):
        # per-core [128 f, TILES*128 node] f32 -> [N, D]
        parts = []
        for c in range(CORES):
            a = res.results[c][key]  # [128, 2560]
            parts.append(np.ascontiguousarray(
                a.reshape(D, TILES, 128).transpose(1, 2, 0).reshape(SHARD, D)))
        return np.concatenate(parts)[:N]

    return unshard("out_my"), unshard("out_opp")


# revision 14
# speedup vs baseline: 3.2183x; 1.0094x over previous
"""Trainium2 Bass kernel for the 2-layer heterogeneous GNN (GATv2 + CGConv).

Sharding: destination nodes (both node types) are split into 8 contiguous
ranges of 2560 (N padded 20000 -> 20480); each core owns the edges that
target its range, for all 4 relations.  Node features are replicated
(SBUF-resident, bf16, node-wrapped layout) so per-edge source gathers are
SBUF->SBUF dma_gather ops (feature-major output); destination-side
per-edge values come from one-hot selector matmuls on the PE.  The
one-hot matrices (static, from the edge lists) are precomputed on the
host and streamed from HBM.  The inter-layer halo exchange is a single
AllGather of the updated 2560-row slices.
"""

import os
import numpy as np
import ml_dtypes

BF = ml_dtypes.bfloat16

N = 20000
D = 128
H = 4
L = 2
E = 80000
CORES = 8
NPAD = 20480
SHARD = 2560
TILES = 20           # dst tiles of 128 per core
RANKS = NPAD // 128  # 160
PAD_NODE = 20000     # zero-feature padding node (valid gather target)
GTILES = 4           # dst tiles per gather chunk

LAST_EXEC_NS = None
DBG = None

# relation table: (name, kind, src_type, dst_type); cg before gat per dst type
RELS = [
    ("loses", "cg", "my", "opp"),
    ("beats", "gat", "my", "opp"),
    ("rev_beats", "cg", "opp", "my"),
    ("rev_loses", "gat", "opp", "my"),
]


# ----------------------------------------------------------------- host prep

def _wrap_nodes(x):
    """[N,128] f32 -> node-wrapped [128, RANKS*128] bf16 (node n at
    partition n%128, cols (n//128)*128 : +128)."""
    xp = np.zeros((NPAD, D), np.float32)
    xp[:N] = x
    return np.ascontiguousarray(
        xp.reshape(RANKS, 128, D).transpose(1, 0, 2).reshape(128, RANKS * D)
    ).astype(BF)


def _dst_major_slice(x, c):
    """core c's own dst slice, dst-major [128, TILES*128] bf16."""
    xp = np.zeros((NPAD, D), np.float32)
    xp[:N] = x
    sl = xp[c * SHARD:(c + 1) * SHARD]
    return np.ascontiguousarray(
        sl.reshape(TILES, 128, D).transpose(1, 0, 2).reshape(128, TILES * D)
    ).astype(BF)


def _feat_major_slice(x, c):
    """core c's own dst slice, feature-major [128, TILES*128] bf16
    (col t*128+j = node c*2560+t*128+j)."""
    xp = np.zeros((NPAD, D), np.float32)
    xp[:N] = x
    sl = xp[c * SHARD:(c + 1) * SHARD]  # [2560, D]
    return np.ascontiguousarray(
        sl.reshape(TILES, 128, D).transpose(2, 0, 1).reshape(D, TILES * 128)
    ).astype(BF)


def _prep_edges(ei):
    """bucket edges by (core, dst tile); returns per-core lists + max count."""
    src = np.asarray(ei[0]).astype(np.int64)
    dst = np.asarray(ei[1]).astype(np.int64)
    percore = []
    maxcnt = 1
    for c in range(CORES):
        m = (dst >= c * SHARD) & (dst < (c + 1) * SHARD)
        s, d = src[m], dst[m]
        dl = d - c * SHARD
        tid = dl // 128
        buckets = [np.nonzero(tid == t)[0] for t in range(TILES)]
        for b in buckets:
            maxcnt = max(maxcnt, len(b))
        percore.append((s, dl, buckets))
    return percore, maxcnt


def _pack_edges(percore, Bmax):
    """-> per-core (src_ids [EP], dloc [EP]); EP = TILES*Bmax*128, pad=-1."""
    out = []
    for (s, dl, buckets) in percore:
        src_a = np.full((TILES, Bmax * 128), PAD_NODE, np.int64)
        loc_a = np.full((TILES, Bmax * 128), -1, np.int64)
        for t, b in enumerate(buckets):
            n = len(b)
            src_a[t, :n] = s[b]
            loc_a[t, :n] = dl[b] % 128
        out.append((src_a.reshape(-1), loc_a.reshape(-1)))
    return out


def _onehots(loc, Bmax):
    """loc [EP] (-1 = pad) -> (oh_e [128, NB*128], oh_d [128, NB*128]) bf16.

    oh_e block gb: [j=edge-in-block, d=dst-local]; oh_d block = transpose."""
    NB = TILES * Bmax
    EP = NB * 128
    M = np.zeros((EP, 128), np.float32)
    valid = loc >= 0
    M[np.nonzero(valid)[0], loc[valid]] = 1.0
    Mb = M.reshape(NB, 128, 128)
    oh_e = np.ascontiguousarray(Mb.transpose(1, 0, 2).reshape(128, NB * 128))
    oh_d = np.ascontiguousarray(Mb.transpose(2, 0, 1).reshape(128, NB * 128))
    return oh_e.astype(BF), oh_d.astype(BF)


def _idx_dev(a):
    """[EP] int -> [128, EP//16] int16 (16-partition wrap, replicated 8x)."""
    x = a.astype(np.int16).reshape(-1, 16).T
    return np.ascontiguousarray(np.tile(x, (8, 1)))


def _rep(v, rows=128):
    return np.ascontiguousarray(
        np.tile(np.asarray(v, np.float32).reshape(1, -1), (rows, 1)))


# ------------------------------------------------------------- program build

def _build_program(Bmax):
    import concourse.bass as bass
    import concourse.bacc as bacc
    import concourse.mybir as mybir
    import concourse.tile as tile
    from concourse.hw_specs import get_activation_tables

    F32, BF16, I16 = mybir.dt.float32, mybir.dt.bfloat16, mybir.dt.int16
    AF = mybir.ActivationFunctionType
    OP = mybir.AluOpType

    NB = TILES * Bmax
    EP = NB * 128
    EPQ = GTILES * Bmax * 128       # idxs per gather chunk
    NCH = TILES // GTILES           # gather chunks per relation
    CB = GTILES * Bmax              # blocks per chunk

    k_layers = int(os.environ.get("K_LAYERS", str(L)))
    k_rels = os.environ.get("K_RELS", "")
    rels_active = [r for r in RELS if (not k_rels or r[0] in k_rels.split(","))]

    nc = bacc.Bacc("TRN2", target_bir_lowering=False, debug=False,
                   num_devices=CORES)

    dr = {}
    dr["xw_my"] = nc.dram_tensor("xw_my", [128, RANKS * D], BF16, kind="ExternalInput")
    dr["xw_opp"] = nc.dram_tensor("xw_opp", [128, RANKS * D], BF16, kind="ExternalInput")
    for ty in ("my", "opp"):
        dr[f"xres_{ty}"] = nc.dram_tensor(f"xres_{ty}", [128, TILES * D], BF16, kind="ExternalInput")
        dr[f"xfm_{ty}"] = nc.dram_tensor(f"xfm_{ty}", [128, TILES * 128], BF16, kind="ExternalInput")
    for rname, kind, _, _ in RELS:
        dr[f"si_{rname}"] = nc.dram_tensor(f"si_{rname}", [128, EP // 16], I16, kind="ExternalInput")
        dr[f"ohe_{rname}"] = nc.dram_tensor(f"ohe_{rname}", [128, NB * 128], BF16, kind="ExternalInput")
        dr[f"ohd_{rname}"] = nc.dram_tensor(f"ohd_{rname}", [128, NB * 128], BF16, kind="ExternalInput")
        if kind == "gat":
            dr[f"wl_{rname}"] = nc.dram_tensor(f"wl_{rname}", [L, 128, H * D], BF16, kind="ExternalInput")
            dr[f"wr_{rname}"] = nc.dram_tensor(f"wr_{rname}", [L, 128, H * D], BF16, kind="ExternalInput")
            dr[f"att_{rname}"] = nc.dram_tensor(f"att_{rname}", [L, 128, H * D], BF16, kind="ExternalInput")
            dr[f"gb_{rname}"] = nc.dram_tensor(f"gb_{rname}", [L, 128, D], F32, kind="ExternalInput")
        else:
            dr[f"wt_{rname}"] = nc.dram_tensor(f"wt_{rname}", [L, 128, 2 * D], BF16, kind="ExternalInput")
            dr[f"wb_{rname}"] = nc.dram_tensor(f"wb_{rname}", [L, 128, 2 * D], BF16, kind="ExternalInput")
            dr[f"cb_{rname}"] = nc.dram_tensor(f"cb_{rname}", [L, 1, 2 * D], BF16, kind="ExternalInput")
    dr["nw_w"] = nc.dram_tensor("nw_w", [L, 128, D], BF16, kind="ExternalInput")
    dr["nw_b"] = nc.dram_tensor("nw_b", [L, 128, 1], F32, kind="ExternalInput")
    dr["ident_f"] = nc.dram_tensor("ident_f", [128, 128], F32, kind="ExternalInput")
    dr["ident_b"] = nc.dram_tensor("ident_b", [128, 128], BF16, kind="ExternalInput")
    dr["out_my"] = nc.dram_tensor("out_my", [128, TILES * D], F32, kind="ExternalOutput")

    dr["out_opp"] = nc.dram_tensor("out_opp", [128, TILES * D], F32, kind="ExternalOutput")

    def ld3(pool, name, src, cols, dt=None):
        t = pool.tile([128, L * cols], dt or src.dtype, name=name, tag=name)
        nc.sync.dma_start(
            t[:].rearrange("p (l n) -> p l n", l=L),
            src[:].rearrange("l p n -> p l n"),
        )
        return t

    with tile.TileContext(nc) as tc:
        with tc.tile_pool(name="const", bufs=1) as cst, \
             tc.tile_pool(name="xwp", bufs=1) as xwp, \
             tc.tile_pool(name="accp", bufs=1) as accp, \
             tc.tile_pool(name="gth", bufs=3) as gth, \
             tc.tile_pool(name="ohp", bufs=2) as ohp, \
             tc.tile_pool(name="wrk", bufs=3) as wrk, \
             tc.tile_pool(name="til", bufs=2) as til, \
             tc.tile_pool(name="epi", bufs=1) as epi, \
             tc.tile_pool(name="dram", bufs=1, space="DRAM") as drm, \
             tc.tile_pool(name="pz", bufs=4, space=bass.MemorySpace.PSUM) as pzp, \
             tc.tile_pool(name="pagg", bufs=2, space=bass.MemorySpace.PSUM) as paggp:

            # one activation table serves Exp/Ln/Prelu/Copy/Identity
            tabs = list(get_activation_tables(nc.m.arch).items())
            need = {AF.Exp, AF.Ln, AF.Prelu, AF.Copy, AF.Identity}
            set_id = next(i for i, (_, fns) in enumerate(tabs) if need <= fns)
            nc.scalar.add_instruction(mybir.InstLoadActFuncSet(
                name=nc.get_next_instruction_name(), ins=[], outs=[],
                act_func_set_id=set_id))

            # ---------------- persistent SBUF state
            xw = {}
            for ty in ("my", "opp"):
                xw[ty] = xwp.tile([128, RANKS * D], BF16, name=f"xw_{ty}_sb", tag=f"xw_{ty}_sb")
                nc.sync.dma_start(xw[ty][:], dr[f"xw_{ty}"][:])
            xres, xfm = {}, {}
            for ty in ("my", "opp"):
                xres[ty] = xwp.tile([128, TILES * D], BF16, name=f"xres_{ty}_sb", tag=f"xres_{ty}_sb")
                nc.sync.dma_start(xres[ty][:], dr[f"xres_{ty}"][:])
                xfm[ty] = xwp.tile([128, TILES * 128], BF16, name=f"xfm_{ty}_sb", tag=f"xfm_{ty}_sb")
                nc.sync.dma_start(xfm[ty][:], dr[f"xfm_{ty}"][:])

            cw = {}
            for rname, kind, _, _ in RELS:
                si = cst.tile([128, EP // 16], I16, name=f"si_{rname}_sb", tag=f"si_{rname}_sb")
                nc.sync.dma_start(si[:], dr[f"si_{rname}"][:])
                cw[rname] = {"si": si}
                if kind == "gat":
                    cw[rname]["wl"] = ld3(cst, f"wl_{rname}_sb", dr[f"wl_{rname}"], H * D)
                    cw[rname]["wr"] = ld3(cst, f"wr_{rname}_sb", dr[f"wr_{rname}"], H * D)
                    cw[rname]["att"] = ld3(cst, f"att_{rname}_sb", dr[f"att_{rname}"], H * D)
                    cw[rname]["gb"] = ld3(cst, f"gb_{rname}_sb", dr[f"gb_{rname}"], D)
                else:
                    cw[rname]["wt"] = ld3(cst, f"wt_{rname}_sb", dr[f"wt_{rname}"], 2 * D)
                    cw[rname]["wb"] = ld3(cst, f"wb_{rname}_sb", dr[f"wb_{rname}"], 2 * D)
                    cbt = cst.tile([1, L * 2 * D], BF16, name=f"cb_{rname}_sb", tag=f"cb_{rname}_sb")
                    nc.sync.dma_start(
                        cbt[:].rearrange("p (l n) -> p l n", l=L),
                        dr[f"cb_{rname}"][:].rearrange("l p n -> p l n"),
                    )
                    cw[rname]["cb"] = cbt
            nw_w = ld3(cst, "nw_w_sb", dr["nw_w"], D)
            nw_b = ld3(cst, "nw_b_sb", dr["nw_b"], 1)
            ident_f = cst.tile([128, 128], F32, name="identf_sb", tag="identf_sb")
            nc.sync.dma_start(ident_f[:], dr["ident_f"][:])
            ident_b = cst.tile([128, 128], BF16, name="identb_sb", tag="identb_sb")
            nc.sync.dma_start(ident_b[:], dr["ident_b"][:])
            ones_b = cst.tile([1, 128], BF16, name="ones_sb", tag="ones_sb")
            nc.gpsimd.memset(ones_b[:], 1.0)

            # ---------------- layers
            def gather_chunk(rname, sty, l, q):
                cwr = cw[rname]
                xs = gth.tile([128, EPQ], BF16, name=f"xs_{rname}_{l}_{q}", tag="xs")
                nc.gpsimd.dma_gather(
                    out_ap=xs[:].rearrange("p (o n) -> p o n", o=1),
                    in_ap=xw[sty][:],
                    idxs_ap=cwr["si"][:, q * (EPQ // 16):(q + 1) * (EPQ // 16)],
                    num_idxs=EPQ, num_idxs_reg=EPQ,
                    elem_size=128, transpose=True,
                    single_packet=False,
                    sbuf_tokens_per_rank=128,
                    sbuf_free_dim_per_rank=256,
                    sbuf_free_dim_pad_per_rank=0,
                    sbuf_byte_offset=0,
                )
                ohe = ohp.tile([128, CB * 128], BF16, name=f"ohe_{rname}_{l}_{q}", tag="ohe")
                nc.sync.dma_start(ohe[:], dr[f"ohe_{rname}"][:, q * CB * 128:(q + 1) * CB * 128])
                ohd = ohp.tile([128, CB * 128], BF16, name=f"ohd_{rname}_{l}_{q}", tag="ohd")
                nc.scalar.dma_start(ohd[:], dr[f"ohd_{rname}"][:, q * CB * 128:(q + 1) * CB * 128])
                return xs, ohe, ohd

            def gat_tile(rname, dty, l, t, tq, ACC, xs, ohe_c, ohd_c):
                cwr = cw[rname]
                pxr = pzp.tile([128, H * D], F32, name=f"pxr_{rname}_{l}_{t}", tag="pz")
                nc.tensor.matmul(pxr[:], xfm[dty][:, t * 128:(t + 1) * 128],
                                 cwr["wr"][:, l * H * D:(l + 1) * H * D],
                                 start=True, stop=True)
                xr_sb = til.tile([128, H * D], BF16, name=f"xrsb_{rname}_{l}_{t}", tag="xr_sb")
                nc.scalar.copy(xr_sb[:], pxr[:])
                pagg = paggp.tile([128, H * D], F32, name=f"pagg_{rname}_{l}_{t}", tag="pagg")
                pden = paggp.tile([128, H], F32, name=f"pden_{rname}_{l}_{t}", tag="pden")

                pend = []
                for b in range(Bmax):
                    off = (tq * Bmax + b) * 128
                    xs_fm = xs[:, off:off + 128]
                    ohe = ohe_c[:, off:off + 128]
                    ohd = ohd_c[:, off:off + 128]
                    first, last = (b == 0), (b == Bmax - 1)
                    # psz_l: Wl part only (for alpha-weighted aggregation)
                    psz_l = pzp.tile([128, H * D], F32, name=f"pszl_{rname}_{l}_{t}_{b}", tag="pz")
                    nc.tensor.matmul(psz_l[:], xs_fm,
                                     cwr["wl"][:, l * H * D:(l + 1) * H * D],
                                     start=True, stop=True)
                    # psz_f: Wl + Wr[dst] (for the score)
                    psz_f = pzp.tile([128, H * D], F32, name=f"pszf_{rname}_{l}_{t}_{b}", tag="pz")
                    nc.tensor.matmul(psz_f[:], ohd, xr_sb[:],
                                     start=True, stop=False)
                    nc.tensor.matmul(psz_f[:], xs_fm,
                                     cwr["wl"][:, l * H * D:(l + 1) * H * D],
                                     start=False, stop=True)
                    z = wrk.tile([128, H * D], BF16, name=f"z_{rname}_{l}_{t}_{b}", tag="z")
                    nc.scalar.activation(z[:], psz_f[:], AF.Prelu, alpha=0.2)
                    scp = wrk.tile([128, H * D], BF16, name=f"scp_{rname}_{l}_{t}_{b}", tag="scp")
                    nc.vector.tensor_tensor(
                        scp[:], z[:],
                        cwr["att"][:, l * H * D:(l + 1) * H * D], op=OP.mult)
                    sc = wrk.tile([128, H], BF16, name=f"sc_{rname}_{l}_{t}_{b}", tag="sc")
                    with nc.allow_low_precision(reason="softmax logits tolerate bf16"):
                        nc.vector.tensor_reduce(
                            sc[:], scp[:].rearrange("p (h f) -> p h f", f=D),
                            axis=mybir.AxisListType.X, op=OP.add)
                    es = wrk.tile([128, H], BF16, name=f"es_{rname}_{l}_{t}_{b}", tag="es")
                    nc.scalar.activation(es[:], sc[:], AF.Exp)
                    # xlw = xl * es[h]  (broadcast along feature dim)
                    xlw = wrk.tile([128, H * D], BF16, name=f"xlw_{rname}_{l}_{t}_{b}", tag="xlw")
                    nc.vector.tensor_tensor(
                        xlw[:].rearrange("p (h f) -> p h f", f=D),
                        psz_l[:].rearrange("p (h f) -> p h f", f=D),
                        es[:].unsqueeze(2).broadcast_to((128, H, D)),
                        op=OP.mult)
                    if pend:
                        pend.pop(0)()
                    pend.append(
                        (lambda ohe=ohe, xlw=xlw, es=es, first=first, last=last: (
                            nc.tensor.matmul(pagg[:], ohe, xlw[:], start=first, stop=last),
                            nc.tensor.matmul(pden[:], ohe, es[:], start=first, stop=last))))
                while pend:
                    pend.pop(0)()

                # -------- tile epilogue (gat adds after cg wrote ACC)
                asl = ACC[dty][:, t * D:(t + 1) * D]
                sden = til.tile([128, H], F32, name=f"sden_{rname}_{l}_{t}", tag="sden")
                nc.vector.tensor_scalar(sden[:], pden[:], 1e-16, 4.0,
                                        op0=OP.add, op1=OP.mult)
                inv4 = til.tile([128, H], F32, name=f"inv4_{rname}_{l}_{t}", tag="inv4")
                nc.vector.reciprocal_approx_fast(inv4[:], sden[:])
                gtmp = til.tile([128, H * D], F32, name=f"gtmp_{rname}_{l}_{t}", tag="gtmp")
                nc.vector.tensor_tensor(
                    gtmp[:].rearrange("p (h f) -> p h f", f=D),
                    pagg[:].rearrange("p (h f) -> p h f", f=D),
                    inv4[:].unsqueeze(2).broadcast_to((128, H, D)),
                    op=OP.mult)
                gt = til.tile([128, D], F32, name=f"gt_{rname}_{l}_{t}", tag="gt")
                nc.vector.tensor_reduce(
                    gt[:], gtmp[:].rearrange("p (h f) -> p f h", f=D),
                    axis=mybir.AxisListType.X, op=OP.add)
                gt2 = til.tile([128, D], F32, name=f"gt2_{rname}_{l}_{t}", tag="gt2")
                nc.vector.scalar_tensor_tensor(
                    gt2[:], gt[:], 1.0, cwr["gb"][:, l * D:(l + 1) * D],
                    op0=OP.mult, op1=OP.add)
                nc.vector.tensor_tensor(asl, asl, gt2[:], op=OP.add)

            def cg_tile(rname, dty, l, t, tq, ACC, xs, ohe_c, ohd_c):
                cwr = cw[rname]
                pud = pzp.tile([128, 2 * D], F32, name=f"pud_{rname}_{l}_{t}", tag="pz")
                nc.tensor.matmul(pud[:], xfm[dty][:, t * 128:(t + 1) * 128],
                                 cwr["wt"][:, l * 2 * D:(l + 1) * 2 * D],
                                 start=True, stop=False)
                nc.tensor.matmul(pud[:], ones_b[:],
                                 cwr["cb"][:, l * 2 * D:(l + 1) * 2 * D],
                                 start=False, stop=True)
                ud_sb = til.tile([128, 2 * D], BF16, name=f"udsb_{rname}_{l}_{t}", tag="ud_sb")
                nc.scalar.copy(ud_sb[:], pud[:])
                pagg = paggp.tile([128, D], F32, name=f"pagg_{rname}_{l}_{t}", tag="pagg")

                pend = []
                for b in range(Bmax):
                    off = (tq * Bmax + b) * 128
                    xs_fm = xs[:, off:off + 128]
                    ohe = ohe_c[:, off:off + 128]
                    ohd = ohd_c[:, off:off + 128]
                    first, last = (b == 0), (b == Bmax - 1)
                    psm = pzp.tile([128, 2 * D], F32, name=f"psm_{rname}_{l}_{t}_{b}", tag="pz")
                    nc.tensor.matmul(psm[:], ohd, ud_sb[:],
                                     start=True, stop=False)
                    nc.tensor.matmul(psm[:], xs_fm,
                                     cwr["wb"][:, l * 2 * D:(l + 1) * 2 * D],
                                     start=False, stop=True)
                    # cols 0:D hold -u; cols D:2D hold v (Wf negated on host)
                    s1 = wrk.tile([128, 2 * D], F32, name=f"s1_{rname}_{l}_{t}_{b}", tag="s1")
                    nc.scalar.activation(s1[:], psm[:], AF.Exp)
                    sp = wrk.tile([128, D], F32, name=f"sp_{rname}_{l}_{t}_{b}", tag="sp")
                    nc.scalar.activation(sp[:], s1[:, D:2 * D], AF.Ln, bias=1.0)
                    d1 = wrk.tile([128, D], F32, name=f"d1_{rname}_{l}_{t}_{b}", tag="d1")
                    nc.scalar.activation(d1[:], s1[:, 0:D], AF.Identity, bias=1.0)
                    rsg = wrk.tile([128, D], F32, name=f"rsg_{rname}_{l}_{t}_{b}", tag="rsg")
                    nc.vector.reciprocal_approx_fast(rsg[:], d1[:])
                    m = wrk.tile([128, D], BF16, name=f"m_{rname}_{l}_{t}_{b}", tag="m")
                    nc.vector.tensor_tensor(m[:], rsg[:], sp[:], op=OP.mult)
                    if len(pend) >= 2:
                        pend.pop(0)()
                    pend.append(
                        (lambda ohe=ohe, m=m, first=first, last=last:
                            nc.tensor.matmul(pagg[:], ohe, m[:], start=first, stop=last)))
                while pend:
                    pend.pop(0)()

                # -------- tile epilogue: ACC = cg_agg + residual (cg first)
                asl = ACC[dty][:, t * D:(t + 1) * D]
                nc.vector.scalar_tensor_tensor(
                    asl, pagg[:], 1.0, xres[dty][:, t * D:(t + 1) * D],
                    op0=OP.mult, op1=OP.add)

            def type_epilogue(ty, tyi, l, ACC, last_layer):
                accT = epi.tile([128, TILES * D], BF16, name=f"accT_{ty}_{l}", tag="accT")
                for t in range(TILES):
                    ptr = pzp.tile([128, 128], BF16, name=f"ptr_{ty}_{l}_{t}", tag="pz")
                    nc.tensor.transpose(ptr[:], ACC[ty][:, t * D:(t + 1) * D], ident_b[:])
                    nc.scalar.copy(accT[:, t * D:(t + 1) * D], ptr[:])
                for k in range(TILES * D // 512):
                    pnw = paggp.tile([128, 512], F32, name=f"pnw_{ty}_{l}_{k}", tag="pagg")
                    nc.tensor.matmul(pnw[:], nw_w[:, l * D:(l + 1) * D],
                                     accT[:, k * 512:(k + 1) * 512],
                                     start=True, stop=True)
                    if last_layer:
                        osb = epi.tile([128, 512], F32, name=f"osb_{ty}_{l}_{k}", tag="osb")
                        nc.scalar.activation(osb[:], pnw[:], AF.Identity,
                                             bias=nw_b[:, l:l + 1])
                        nc.sync.dma_start(dr[f"out_{ty}"][:, k * 512:(k + 1) * 512], osb[:])
                    else:
                        nc.scalar.activation(xfm[ty][:, k * 512:(k + 1) * 512], pnw[:],
                                             AF.Identity, bias=nw_b[:, l:l + 1])
                if not last_layer:
                    # back to dst-major for residuals + halo exchange
                    for t in range(TILES):
                        ptr2 = pzp.tile([128, 128], BF16, name=f"ptr2_{ty}_{l}_{t}", tag="pz")
                        nc.tensor.transpose(ptr2[:], xfm[ty][:, t * D:(t + 1) * D], ident_b[:])
                        nc.vector.tensor_copy(xres[ty][:, t * D:(t + 1) * D], ptr2[:])
                    ag_in = drm.tile([128, TILES * D], BF16, name=f"agin_{ty}_{l}", tag=f"agin_{ty}")
                    ag_out = drm.tile([CORES * 128, TILES * D], BF16,
                                      name=f"agout_{ty}_{l}", tag=f"agout_{ty}",
                                      addr_space="Shared")
                    nc.sync.dma_start(ag_in[:], xres[ty][:])
                    agins[ty] = (ag_in, ag_out)

            agins = {}
            for l in range(k_layers):
                last_layer = (l == k_layers - 1)
                ACC = {}
                for ty in ("my", "opp"):
                    ACC[ty] = accp.tile([128, TILES * D], BF16, name=f"acc_{ty}_{l}", tag=f"acc_{ty}")

                for rname, kind, sty, dty in rels_active:
                    chunks = [gather_chunk(rname, sty, l, q) for q in range(NCH)]
                    for t in range(TILES):
                        q, tq = t // GTILES, t % GTILES
                        if kind == "cg":
                            cg_tile(rname, dty, l, t, tq, ACC, *chunks[q])
                        else:
                            gat_tile(rname, dty, l, t, tq, ACC, *chunks[q])

                for tyi, ty in enumerate(("my", "opp")):
                    if ty not in {r[3] for r in rels_active}:
                        continue
                    type_epilogue(ty, tyi, l, ACC, last_layer)
                if not last_layer:
                    for ty in ("my", "opp"):
                        ag_in, ag_out = agins.pop(ty)
                        nc.gpsimd.collective_compute(
                            "AllGather", mybir.AluOpType.bypass,
                            replica_groups=[list(range(CORES))],
                            ins=[ag_in.opt()], outs=[ag_out.opt()],
                        )
                        nc.sync.dma_start(
                            xw[ty][:].rearrange("p (c j) -> p c j", c=CORES),
                            ag_out[:].rearrange("(c p) j -> p c j", p=128),
                        )


    nc.compile()
    return nc


_prog_cache = {}


def _get_program(Bmax):
    if Bmax not in _prog_cache:
        _prog_cache[Bmax] = _build_program(Bmax)
    return _prog_cache[Bmax]


# ------------------------------------------------------------------- kernel

def kernel(**inputs):
    global LAST_EXEC_NS
    from concourse.bass_utils import run_bass_kernel_spmd

    f32 = lambda k: np.asarray(inputs[k], np.float32)
    x_my, x_opp = f32("x_my"), f32("x_opp")

    # edges
    eprep = {}
    Bmax = 1
    for rname, key in (("loses", "ei_loses"), ("beats", "ei_beats"),
                       ("rev_beats", "ei_rev_beats"), ("rev_loses", "ei_rev_loses")):
        percore, mc = _prep_edges(np.asarray(inputs[key]))
        eprep[rname] = percore
        Bmax = max(Bmax, -(-mc // 128))
    packed = {r: _pack_edges(eprep[r], Bmax) for r in eprep}

    nc = _get_program(Bmax)

    # shared (per-core identical) tensors
    shared = {}
    shared["xw_my"] = _wrap_nodes(x_my)
    shared["xw_opp"] = _wrap_nodes(x_opp)
    for rname, kind, _, _ in RELS:
        tag = {"loses": "cg_lose", "beats": "gat_beats",
               "rev_beats": "cg_rev", "rev_loses": "gat_rev"}[rname]
        if kind == "gat":
            shared[f"wl_{rname}"] = np.ascontiguousarray(f32(f"{tag}_Wl")).astype(BF)
            shared[f"wr_{rname}"] = np.ascontiguousarray(f32(f"{tag}_Wr")).astype(BF)
            att = f32(f"{tag}_att")  # [L, H, D]
            shared[f"att_{rname}"] = np.stack(
                [_rep(att[l].reshape(-1)) for l in range(L)]).astype(BF)
            b = f32(f"{tag}_b")  # [L, D]
            shared[f"gb_{rname}"] = np.stack([_rep(b[l]) for l in range(L)])
        else:
            # f-gate (Wf) negated so psm[:, :D] = -u and sigmoid(u) = 1/(1+e^{psm0})
            wf, ws = -f32(f"{tag}_Wf"), f32(f"{tag}_Ws")  # [L, 2D, D]
            shared[f"wt_{rname}"] = np.ascontiguousarray(
                np.concatenate([wf[:, :D, :], ws[:, :D, :]], axis=2)).astype(BF)
            shared[f"wb_{rname}"] = np.ascontiguousarray(
                np.concatenate([wf[:, D:, :], ws[:, D:, :]], axis=2)).astype(BF)
            bfv, bsv = -f32(f"{tag}_bf"), f32(f"{tag}_bs")  # [L, D]
            shared[f"cb_{rname}"] = np.ascontiguousarray(
                np.concatenate([bfv, bsv], axis=1).reshape(L, 1, 2 * D)).astype(BF)
    shared["nw_w"] = np.ascontiguousarray(f32("nw_W")).astype(BF)
    shared["nw_b"] = np.ascontiguousarray(f32("nw_b").reshape(L, 128, 1))
    shared["ident_f"] = np.eye(128, dtype=np.float32)
    shared["ident_b"] = np.eye(128).astype(BF)

    in_maps = []
    for c in range(CORES):
        m = dict(shared)
        m["xres_my"] = _dst_major_slice(x_my, c)
        m["xres_opp"] = _dst_major_slice(x_opp, c)
        m["xfm_my"] = _feat_major_slice(x_my, c)
        m["xfm_opp"] = _feat_major_slice(x_opp, c)
        for rname in packed:
            s_a, l_a = packed[rname][c]
            m[f"si_{rname}"] = _idx_dev(s_a)
            ohe, ohd = _onehots(l_a, Bmax)
            m[f"ohe_{rname}"] = ohe
            m[f"ohd_{rname}"] = ohd
        in_maps.append(m)

    trace = os.environ.get("KERNEL_PROFILE", "0") == "1"
    res = run_bass_kernel_spmd(nc, in_maps, core_ids=list(range(CORES)),
                               trace=trace, trace_cores=[0] if trace else None)
    LAST_EXEC_NS = res.exec_time_ns

    global DBG
    DBG = res.results

    def unshard(key):
        # per-core [128 f, TILES*128 node] f32 -> [N, D]
        parts = []
        for c in range(CORES):
            a = res.results[c][key]  # [128, 2560]
            parts.append(np.ascontiguousarray(
                a.reshape(D, TILES, 128).transpose(1, 2, 0).reshape(SHARD, D)))
        return np.concatenate(parts)[:N]

    return unshard("out_my"), unshard("out_opp")
